# revision 17
# baseline (speedup 1.0000x reference)
"""Trainium2 Bass kernel for nn_LocalEnergyOpt (molecular-mechanics local energy).

Per batch sample (B=128): features[:, :, 5] packs coords [4096, 3]; col 6 bonds
(i,j,t)x4095; col 7 angles (i,j,k,t)x4094; col 8 torsions (i,j,k,l,t)x4093.
  e_bond = opt[0] * sum k_t (|ci-cj| - r0_t)^2
  e_ang  = opt[1] * sum k_t (theta - th0_t)^2, theta = arccos(clip(cos))
  e_tor  = opt[2] * sum k_t (1 + cos(n_t phi - d_t)), phi = atan2(y, x)
Output [B, 3].

Sharding: pure data parallel, 16 samples per NeuronCore across 8 cores.

All inputs are packed host-side into ONE dram tensor per core (per-execute
dispatch cost scales with input-buffer count): rows 0..15 are the flat
per-sample features (padded to 128*1449), row 16 is a constant block holding
the replicated param tables, tail masks, reduction selector and opt_pars.

Device pipeline per NC (2 waves x 8 samples; GPSIMD Q7 core c handles sample
8w+c on partitions 16c..16c+15):
  stage features flat -> extract packed columns (stride-9 DVE copies) ->
  dense per-sample DRAM scratch -> read back as (a) a per-partition-replicated
  coords table for ap_gather, (b) dense [128, X] index blocks -> int16
  wrap-layout index lists -> ap_gather endpoint coords -> dedup the
  16x-replicated gather outputs via a DRAM round trip -> dense [128, 256]-col
  DVE/ACT energy pipeline.

Per-edge type params for bonds/angles are NOT gathered (ap_gather costs
~15ns/idx on the Q7s and is the kernel bottleneck); instead per-type masked
sums on DVE use sum k_t (x - x0_t)^2 = k_t (S2 - 2 x0_t S1 + x0_t^2 S0) with
S0/S1/S2 accumulated per type via tensor_tensor_reduce. Torsions keep a d=4
param gather (k, cos d, sin d, n) - 25 types x 7 masked reductions would cost
more DVE than the gather costs GPSIMD.

Torsion angle avoids arccos/atan2 LUTs: cos(phi), sin(phi) are formed by
normalizing (x, y) = (n1.n2, (n1 x b2).n2 / |b2|), and cos(n phi - d) expands
via Chebyshev doubling/tripling + per-type (cos d, sin d) tables.
"""

import os
import sys
import functools

import numpy as np

ABL_NO_GATHER = bool(int(os.environ.get("ABL_NO_GATHER", "0")))
ABL_NO_ENERGY = bool(int(os.environ.get("ABL_NO_ENERGY", "0")))
ABL_NO_TYPES = bool(int(os.environ.get("ABL_NO_TYPES", "0")))

sys.path.insert(0, "/opt/trn_rl_repo")

from concourse import bacc, mybir  # noqa: E402
import concourse.tile as tile  # noqa: E402
from concourse.alu_op_type import AluOpType as Op  # noqa: E402

F32 = mybir.dt.float32
I16 = mybir.dt.int16
I32 = mybir.dt.int32
AF = mybir.ActivationFunctionType
AX = mybir.AxisListType

# Problem constants
N_CORES = 8
NS = 16                      # samples per NeuronCore
NB, NA, NT = 4095, 4094, 4093
NATOMS = 4096
MAXLEN = 20465
LPP = 1449                   # flat f32 per partition (multiple of 9)
FLATPAD = 128 * LPP          # 185472 >= 184185
CR = LPP // 9                # 161 col rows per partition
COLN = 128 * CR              # 20608 dense col length
EPS = 1e-8
PI = float(np.pi)

LIST = 4096                  # per-core index list length per class (padded)
GCH = 1024                   # ap_gather chunk
DP = LIST // 16              # 256 dense positions per partition

# const-row per-partition float layout
C_MT = 0                     # [256] torsion valid mask (1 valid / 0 tail)
C_SHB = 256                  # [256] bond type tail shift (0 / 999)
C_SHA = 512                  # [256] angle type tail shift (0 / 999)
C_BT = 768                   # [30] bond_type (k, r0) x 15
C_AT = 798                   # [26] angle_type (k, th0) x 13
C_TR = 824                   # [50] tor_type (k, delta) x 25
C_MU = 874                   # [25] multiplicity (f32)
C_OP = 899                   # [3] opt_pars[0:3]
C_BLK = 902                  # [8] PE group selector row (p//16 == c)
C_TV = 910                   # [25] type values 0..24 (for batched is_equal)
C_END = 935


def build_nc():
    nc = bacc.Bacc(None, target_bir_lowering=False, debug=False)

    feat = nc.dram_tensor("features", [NS + 1, FLATPAD], F32, kind="ExternalInput")
    out_d = nc.dram_tensor("out", [NS, 3], F32, kind="ExternalOutput")

    with tile.TileContext(nc) as tc:
        with (
            tc.tile_pool(name="table", bufs=1) as tablep,
            tc.tile_pool(name="idx16", bufs=2) as idx16p,
            tc.tile_pool(name="gath", bufs=2) as gathp,
            tc.tile_pool(name="const", bufs=1) as constp,
            tc.tile_pool(name="stage", bufs=1) as stagep,
            tc.tile_pool(name="cext", bufs=2) as cextp,
            tc.tile_pool(name="idxraw", bufs=2) as idxrawp,
            tc.tile_pool(name="dense", bufs=4) as densep,
            tc.tile_pool(name="tcol", bufs=2) as tcolp,
            tc.tile_pool(name="work", bufs=1) as workp,
            tc.tile_pool(name="accp", bufs=1) as accp,
            tc.tile_pool(name="psum", bufs=1, space="PSUM") as psump,
            tc.tile_pool(name="dram", bufs=2, space="DRAM") as dramp,
            tc.tile_pool(name="dramded", bufs=6, space="DRAM") as dedp,
        ):
            # ---------------- constants ------------------------------------
            cst = constp.tile([128, C_END], F32)
            nc.sync.dma_start(
                out=cst[:],
                in_=feat.ap()[NS].rearrange("(p f) -> p f", f=LPP)[:, 0:C_END],
            )
            mT = cst[:, C_MT:C_MT + DP]
            shB = cst[:, C_SHB:C_SHB + DP]
            shA = cst[:, C_SHA:C_SHA + DP]
            btab = cst[:, C_BT:C_BT + 30]
            atab = cst[:, C_AT:C_AT + 26]
            traw = cst[:, C_TR:C_TR + 50]
            multf = cst[:, C_MU:C_MU + 25]
            opt3 = cst[:, C_OP:C_OP + 3]
            blk = cst[:, C_BLK:C_BLK + 8]
            tv25 = cst[:, C_TV:C_TV + 25]

            fwork = constp.tile([128, 136], F32)
            cb = fwork[:, 0:4]
            nc.vector.memset(cb[:, 0:1], EPS)
            nc.vector.memset(cb[:, 1:2], PI / 2.0)
            nc.vector.memset(cb[:, 2:3], 1e-30)
            nc.vector.memset(cb[:, 3:4], -1.0)
            b_eps = cb[:, 0:1]
            b_pi2 = cb[:, 1:2]
            b_tiny = cb[:, 2:3]
            s_neg1 = cb[:, 3:4]

            # torsion derived table (k, cos d, sin d, n) x 25, on all parts
            ttab = fwork[:, 4:104]
            t4v = ttab.rearrange("p (n d) -> p n d", d=4)
            trv = traw.rearrange("p (n d) -> p n d", d=2)
            nc.vector.tensor_copy(out=t4v[:, :, 0], in_=trv[:, :, 0])          # k
            # cos d = sin(pi/2 - d); d in [0, 3.15) keeps the arg in [-pi, pi]
            carg = fwork[:, 110:135]
            nc.vector.tensor_scalar(out=carg, in0=trv[:, :, 1], scalar1=-1.0,
                                    scalar2=PI / 2.0, op0=Op.mult, op1=Op.add)
            nc.scalar.activation(t4v[:, :, 1], carg, AF.Sin)
            nc.scalar.activation(t4v[:, :, 2], trv[:, :, 1], AF.Sin)           # sin d
            nc.vector.tensor_copy(out=t4v[:, :, 3], in_=multf)                 # n

            opt6 = fwork[0:8, 104:110]
            nc.vector.tensor_copy(out=opt6[:, 0:3], in_=opt3[0:8, :])
            nc.vector.tensor_copy(out=opt6[:, 3:6], in_=opt3[0:8, :])

            accb = accp.tile([128, DP + 6 + 8], F32)
            scr = accb[:, 0:DP]            # TTR mandatory elementwise out
            acc6 = accb[:, DP:DP + 6]
            otmp = accb[0:8, DP + 6:DP + 12]
            rtmp = accb[:, DP + 12:DP + 13]

            btv = btab.rearrange("p (n d) -> p n d", d=2)
            atv = atab.rearrange("p (n d) -> p n d", d=2)
            ttab4 = ttab.rearrange("p (n d) -> p n d", d=4)

            # ========== PREP PHASE: both waves' staging + index prep =====
            # (emitted before any energy math so wave-1 gather prerequisites
            # are not queued behind wave-0 energy on the DVE)
            wave_state = []
            for w in range(2):
                cols_s = dramp.tile([32, COLN], F32, tag="cols_s")
                # row 4*s8 + c holds class c (coords/bonds/angs/tors) of s8
                for s8 in range(8):
                    s = 8 * w + s8
                    stage = stagep.tile([128, LPP], F32, tag="stage")
                    nc.sync.dma_start(
                        out=stage[:], in_=feat.ap()[s].rearrange("(p f) -> p f", f=LPP)
                    )
                    stv = stage[:].rearrange("p (r n) -> p r n", n=9)
                    cd = cextp.tile([128, 4 * CR], F32, tag="cd")
                    cdv = cd[:].rearrange("p (c r) -> p c r", r=CR)
                    for k, col in enumerate((5, 6, 7, 8)):
                        nc.vector.tensor_copy(out=cdv[:, k, :], in_=stv[:, :, col])
                    nc.sync.dma_start(
                        out=cols_s[:][4 * s8:4 * (s8 + 1)].rearrange(
                            "c (p r) -> p c r", r=CR),
                        in_=cd[:].rearrange("p (c r) -> p c r", r=CR),
                    )
                cols_v = cols_s[:].rearrange("(a c) f -> a c f", c=4)

                iraw = idxrawp.tile([128, 3072], F32, tag="iraw")
                braw = iraw[:, 0:768]
                araw = iraw[:, 768:1792]
                trawi = iraw[:, 1792:3072]
                nc.sync.dma_start(
                    out=braw,
                    in_=cols_v[:, 1, 0:12288].rearrange("a (j f) -> a j f", j=16),
                )
                nc.sync.dma_start(
                    out=araw,
                    in_=cols_v[:, 2, 0:16384].rearrange("a (j f) -> a j f", j=16),
                )
                nc.sync.dma_start(
                    out=trawi,
                    in_=cols_v[:, 3, 0:20480].rearrange("a (j f) -> a j f", j=16),
                )

                idxt = idx16p.tile([128, 10 * 256], I16, tag="idxt")
                bv = braw.rearrange("p (e k) -> p e k", k=3)
                av = araw.rearrange("p (e k) -> p e k", k=4)
                tv = trawi.rearrange("p (e k) -> p e k", k=5)
                for k in range(2):
                    nc.vector.tensor_copy(out=idxt[:, 256 * k:256 * (k + 1)],
                                          in_=bv[:, :, k])
                for k in range(3):
                    nc.vector.tensor_copy(out=idxt[:, 256 * (2 + k):256 * (3 + k)],
                                          in_=av[:, :, k])
                for k in range(5):
                    nc.vector.tensor_copy(out=idxt[:, 256 * (5 + k):256 * (6 + k)],
                                          in_=tv[:, :, k])
                # lists: 0,1 = bond i,j; 2..4 = angle i,j,k; 5..9 = tor i,j,k,l,t

                tcols = tcolp.tile([128, 2 * DP], F32, tag="tcols")
                tB = tcols[:, 0:DP]
                tA = tcols[:, DP:2 * DP]
                nc.vector.tensor_tensor(out=tB, in0=bv[:, :, 2], in1=shB, op=Op.add)
                nc.vector.tensor_tensor(out=tA, in0=av[:, :, 3], in1=shA, op=Op.add)
                wave_state.append((cols_v, idxt, tB, tA))

            # ========== COMPUTE PHASE: per-wave gathers + energy ==========
            for w in range(2):
                cols_v, idxt, tB, tA = wave_state[w]

                def idx_list(n):
                    return idxt[:, 256 * n:256 * (n + 1)]

                # gather table (replicated coords): partition p holds sample
                # (p//16)'s coords; step-0 src AP replicates each sample's
                # row across its 16 partitions
                table = tablep.tile([128, 3 * NATOMS], F32, tag="table")
                nc.sync.dma_start(
                    out=table[:],
                    in_=cols_v[:, 0, None, 0:3 * NATOMS].to_broadcast(
                        [8, 16, 3 * NATOMS]),
                )

                def gather_dedup(idx_n, tab_ap, n_elems, d):
                    """ap_gather LIST indices; dedup 16x replicas via DRAM.
                    Returns dense [128, DP*d] tile (edge position 256j+u on
                    partition 16c+j at cols u*d..). Chunk size keeps every
                    call inside the same 24KB/partition pool slot."""
                    gch = 2048 if d == 3 else 1024
                    ded = dedp.tile([8, LIST * d], F32, tag="ded")
                    for ch in range(LIST // gch):
                        g = gathp.tile([128, gch * d], F32, tag="g", name="g")
                        if ABL_NO_GATHER:
                            nc.vector.memset(g[0:8, 0:1], 0.0)
                        else:
                            nc.gpsimd.ap_gather(
                                out_ap=g[:].rearrange("p (n d) -> p n d", d=d),
                                in_ap=tab_ap,
                                idxs_ap=idx_list(idx_n)[:, (gch // 16) * ch:(gch // 16) * (ch + 1)],
                                channels=128,
                                num_elems=n_elems,
                                d=d,
                                num_idxs=gch,
                            )
                        nc.sync.dma_start(
                            out=ded[:][:, gch * d * ch:gch * d * (ch + 1)],
                            in_=g[:].rearrange("(a b) f -> a b f", b=16)[:, 0, :],
                        )
                    dn = densep.tile([128, DP * d], F32, tag="dense")
                    ded_flat = ded[:].rearrange("s f -> (s f)")
                    nc.sync.dma_start(
                        out=dn[:],
                        in_=ded_flat.rearrange("(p f) -> p f", f=DP * d),
                    )
                    return dn

                tab3 = table[:].rearrange("p (n d) -> p n d", d=3)

                acc = acc6[:, 3 * w:3 * w + 3]
                nc.vector.memset(acc, 0.0)

                def type_masked_energy(xcol, tcol, ptab, n_types, acc_col):
                    """acc_col += sum_t k_t sum_{e: t(e)=t} (x_e - x0_t)^2,
                    processed 4 types per batch with broadcast APs."""
                    if ABL_NO_TYPES:
                        nc.vector.tensor_reduce(out=rtmp, in_=xcol, axis=AX.X, op=Op.add)
                        nc.vector.tensor_add(out=acc_col, in0=acc_col, in1=rtmp)
                        return
                    st = workp.tile([128, 32], F32, tag="wst")
                    S = st[:, 0:n_types]
                    wide = workp.tile([128, 9 * DP], F32, tag="w9")
                    for t0 in range(0, n_types, 4):
                        nb = min(4, n_types - t0)
                        msk = wide[:, 0:nb * DP].rearrange(
                            "p (t e) -> p t e", e=DP)
                        dif = wide[:, 4 * DP:(4 + nb) * DP].rearrange(
                            "p (t e) -> p t e", e=DP)
                        nc.vector.tensor_tensor(
                            out=msk,
                            in0=tcol[:, None, :].to_broadcast([128, nb, DP]),
                            in1=tv25[:, t0:t0 + nb][:, :, None].to_broadcast(
                                [128, nb, DP]),
                            op=Op.is_equal)
                        nc.vector.tensor_tensor(
                            out=dif,
                            in0=xcol[:, None, :].to_broadcast([128, nb, DP]),
                            in1=ptab[:, t0:t0 + nb, 1][:, :, None].to_broadcast(
                                [128, nb, DP]),
                            op=Op.subtract)
                        nc.scalar.activation(dif, dif, AF.Square)
                        nc.vector.tensor_tensor(out=msk, in0=msk, in1=dif,
                                                op=Op.mult)
                        nc.vector.tensor_reduce(
                            out=S[:, t0:t0 + nb], in_=msk, axis=AX.X, op=Op.add)
                    cmb = workp.tile([128, 32], F32, tag="wcmb")
                    c1 = cmb[:, 0:n_types]
                    nc.vector.tensor_tensor(out=c1, in0=S, in1=ptab[:, 0:n_types, 0],
                                            op=Op.mult)
                    nc.vector.tensor_reduce(out=rtmp, in_=c1, axis=AX.X, op=Op.add)
                    nc.vector.tensor_add(out=acc_col, in0=acc_col, in1=rtmp)

                # ==================== BONDS ====================
                ci = gather_dedup(0, tab3, NATOMS, 3)
                cj = gather_dedup(1, tab3, NATOMS, 3)
                d3 = workp.tile([128, 3 * DP], F32, tag="w3a")
                nc.vector.tensor_sub(out=d3[:], in0=ci[:], in1=cj[:])
                d3s = workp.tile([128, 3 * DP], F32, tag="w3b")
                nc.vector.tensor_mul(out=d3s[:], in0=d3[:], in1=d3[:])
                wb = workp.tile([128, 8 * DP], F32, tag="w8")
                r2 = wb[:, 0:DP]
                nc.vector.tensor_reduce(
                    out=r2, in_=d3s[:].rearrange("p (n d) -> p n d", d=3),
                    axis=AX.X, op=Op.add,
                )
                r = wb[:, DP:2 * DP]
                nc.scalar.activation(r, r2, AF.Sqrt, bias=b_eps)
                type_masked_energy(r, tB, btv, 15, acc[:, 0:1])

                # ==================== ANGLES ====================
                gi = gather_dedup(2, tab3, NATOMS, 3)
                gj = gather_dedup(3, tab3, NATOMS, 3)
                gk = gather_dedup(4, tab3, NATOMS, 3)
                v1 = workp.tile([128, 3 * DP], F32, tag="w3a")
                v2 = workp.tile([128, 3 * DP], F32, tag="w3b")
                nc.vector.tensor_sub(out=v1[:], in0=gi[:], in1=gj[:])
                nc.vector.tensor_sub(out=v2[:], in0=gk[:], in1=gj[:])
                prod = workp.tile([128, 3 * DP], F32, tag="w3c")
                wa = workp.tile([128, 8 * DP], F32, tag="w8")
                d11 = wa[:, 0:DP]
                d22 = wa[:, 1 * DP:2 * DP]
                d12 = wa[:, 2 * DP:3 * DP]

                def dot3(dst, a, b):
                    nc.vector.tensor_mul(out=prod[:], in0=a[:], in1=b[:])
                    nc.vector.tensor_reduce(
                        out=dst, in_=prod[:].rearrange("p (n d) -> p n d", d=3),
                        axis=AX.X, op=Op.add,
                    )

                dot3(d11, v1, v1)
                dot3(d22, v2, v2)
                dot3(d12, v1, v2)
                s1 = wa[:, 3 * DP:4 * DP]
                s2a = wa[:, 4 * DP:5 * DP]
                nc.scalar.activation(s1, d11, AF.Sqrt, bias=b_eps)
                nc.scalar.activation(s2a, d22, AF.Sqrt, bias=b_eps)
                den = wa[:, 5 * DP:6 * DP]
                nc.vector.tensor_mul(out=den, in0=s1, in1=s2a)
                cosv = wa[:, 6 * DP:7 * DP]
                nc.vector.reciprocal(out=den, in_=den)
                nc.vector.tensor_mul(out=cosv, in0=d12, in1=den)
                cosc = wa[:, 7 * DP:8 * DP]
                nc.vector.tensor_scalar(
                    out=cosc, in0=cosv, scalar1=-1.0 + 1e-6, scalar2=1.0 - 1e-6,
                    op0=Op.max, op1=Op.min,
                )
                # theta = arccos(cosc) via two bounded-arg arctan branches
                # (ACT Arctan domain is [-pi/2, pi/2] so |arg| <= 1 required):
                #  |c| >  s: theta = arctan(s/c) + pi*(c<0)
                #  |c| <= s: theta = pi/2 - arctan(c/s), s = sqrt(1-c^2)
                cc = wa[:, 0:DP]                       # d11 dead
                nc.scalar.activation(cc, cosc, AF.Square)
                om = wa[:, 1 * DP:2 * DP]              # d22 dead
                nc.vector.tensor_scalar(
                    out=om, in0=cc, scalar1=-1.0, scalar2=1.0, op0=Op.mult, op1=Op.add
                )
                sn = wa[:, 2 * DP:3 * DP]              # d12 dead
                nc.scalar.activation(sn, om, AF.Sqrt)
                sgn = wa[:, 3 * DP:4 * DP]             # s1 dead
                nc.vector.tensor_scalar(
                    out=sgn, in0=cosc, scalar1=0.0, scalar2=None, op0=Op.is_ge)
                nc.vector.tensor_scalar(
                    out=sgn, in0=sgn, scalar1=2e-18, scalar2=-1e-18,
                    op0=Op.mult, op1=Op.add)
                csafe = wa[:, 4 * DP:5 * DP]           # s2a dead
                nc.vector.tensor_add(out=csafe, in0=cosc, in1=sgn)
                ra = wa[:, 3 * DP:4 * DP]              # sgn dead
                nc.vector.reciprocal(out=csafe, in_=csafe)
                nc.vector.tensor_mul(out=ra, in0=sn, in1=csafe)
                nc.vector.tensor_scalar(
                    out=ra, in0=ra, scalar1=-1.0, scalar2=1.0, op0=Op.max, op1=Op.min)
                ata = wa[:, 4 * DP:5 * DP]             # csafe dead
                nc.scalar.activation(ata, ra, AF.Arctan)
                corr = wa[:, 5 * DP:6 * DP]            # den dead
                nc.vector.tensor_scalar(
                    out=corr, in0=cosc, scalar1=0.0, scalar2=PI, op0=Op.is_lt, op1=Op.mult
                )
                tha = wa[:, 3 * DP:4 * DP]             # ra dead
                nc.vector.tensor_add(out=tha, in0=ata, in1=corr)
                rb = wa[:, 4 * DP:5 * DP]              # ata dead
                nc.vector.reciprocal(out=sn, in_=sn)
                nc.vector.tensor_mul(out=rb, in0=cosc, in1=sn)
                nc.vector.tensor_scalar(
                    out=rb, in0=rb, scalar1=-1.0, scalar2=1.0, op0=Op.max, op1=Op.min)
                thb = wa[:, 5 * DP:6 * DP]             # corr dead
                nc.scalar.activation(thb, rb, AF.Arctan)
                nc.vector.tensor_scalar(
                    out=thb, in0=thb, scalar1=-1.0, scalar2=PI / 2.0,
                    op0=Op.mult, op1=Op.add)
                wi2 = workp.tile([128, DP], I32, tag="wi")
                mbr = wi2[:, 0:DP]
                nc.vector.tensor_scalar(
                    out=mbr, in0=cc, scalar1=0.5, scalar2=None, op0=Op.is_gt)
                th = wa[:, 6 * DP:7 * DP]              # cosv dead
                nc.vector.select(out=th, mask=mbr, on_true=tha, on_false=thb)
                type_masked_energy(th, tA, atv, 13, acc[:, 1:2])

                # ==================== TORSIONS ====================
                ti = gather_dedup(5, tab3, NATOMS, 3)
                tj = gather_dedup(6, tab3, NATOMS, 3)
                tk_ = gather_dedup(7, tab3, NATOMS, 3)
                tl = gather_dedup(8, tab3, NATOMS, 3)
                b1 = workp.tile([128, 3 * DP], F32, tag="w3a")
                b2 = workp.tile([128, 3 * DP], F32, tag="w3b")
                b3 = workp.tile([128, 3 * DP], F32, tag="w3c")
                nc.vector.tensor_sub(out=b1[:], in0=tj[:], in1=ti[:])
                nc.vector.tensor_sub(out=b2[:], in0=tk_[:], in1=tj[:])
                nc.vector.tensor_sub(out=b3[:], in0=tl[:], in1=tk_[:])
                pt = gather_dedup(9, ttab4, 25, 4)
                pl = workp.tile([128, 9 * DP], F32, tag="w9")

                def plv(n):
                    return pl[:, DP * n:DP * (n + 1)]

                for m in range(3):
                    nc.vector.tensor_copy(
                        out=plv(0 + m),
                        in_=b1[:].rearrange("p (n d) -> p n d", d=3)[:, :, m])
                    nc.vector.tensor_copy(
                        out=plv(3 + m),
                        in_=b2[:].rearrange("p (n d) -> p n d", d=3)[:, :, m])
                    nc.vector.tensor_copy(
                        out=plv(6 + m),
                        in_=b3[:].rearrange("p (n d) -> p n d", d=3)[:, :, m])
                # n1 = b1 x b2 -> cr 0..2 ; n2 = b2 x b3 -> cr 3..5
                cr_ = workp.tile([128, 6 * DP], F32, tag="w6")

                def crv(n):
                    return cr_[:, DP * n:DP * (n + 1)]

                tmp = workp.tile([128, 2 * DP], F32, tag="w2")
                t0 = tmp[:, 0:DP]
                t1_ = tmp[:, DP:2 * DP]
                for m in range(3):
                    mp1, mp2 = (m + 1) % 3, (m + 2) % 3
                    nc.vector.tensor_mul(out=t0, in0=plv(0 + mp1), in1=plv(3 + mp2))
                    nc.vector.tensor_mul(out=t1_, in0=plv(0 + mp2), in1=plv(3 + mp1))
                    nc.vector.tensor_sub(out=crv(m), in0=t0, in1=t1_)
                    nc.vector.tensor_mul(out=t0, in0=plv(3 + mp1), in1=plv(6 + mp2))
                    nc.vector.tensor_mul(out=t1_, in0=plv(3 + mp2), in1=plv(6 + mp1))
                    nc.vector.tensor_sub(out=crv(3 + m), in0=t0, in1=t1_)
                wt = workp.tile([128, 8 * DP], F32, tag="w8")
                q2 = wt[:, 0:DP]
                nc.vector.tensor_mul(out=b1[:], in0=b2[:], in1=b2[:])  # b1 = scratch
                nc.vector.tensor_reduce(
                    out=q2, in_=b1[:].rearrange("p (n d) -> p n d", d=3),
                    axis=AX.X, op=Op.add,
                )
                # m1' = n1 x b2 (normalization folded into rn)
                mp = workp.tile([128, 3 * DP], F32, tag="w3a")

                def mpv(n):
                    return mp[:, DP * n:DP * (n + 1)]

                for m in range(3):
                    mp1, mp2 = (m + 1) % 3, (m + 2) % 3
                    nc.vector.tensor_mul(out=t0, in0=crv(mp1), in1=plv(3 + mp2))
                    nc.vector.tensor_mul(out=t1_, in0=crv(mp2), in1=plv(3 + mp1))
                    nc.vector.tensor_sub(out=mpv(m), in0=t0, in1=t1_)
                X = wt[:, 1 * DP:2 * DP]
                Y = wt[:, 2 * DP:3 * DP]
                nc.vector.tensor_mul(out=t0, in0=crv(0), in1=crv(3))
                nc.vector.tensor_mul(out=t1_, in0=crv(1), in1=crv(4))
                nc.vector.tensor_add(out=X, in0=t0, in1=t1_)
                nc.vector.tensor_mul(out=t0, in0=crv(2), in1=crv(5))
                nc.vector.tensor_add(out=X, in0=X, in1=t0)
                nc.vector.tensor_mul(out=t0, in0=mpv(0), in1=crv(3))
                nc.vector.tensor_mul(out=t1_, in0=mpv(1), in1=crv(4))
                nc.vector.tensor_add(out=Y, in0=t0, in1=t1_)
                nc.vector.tensor_mul(out=t0, in0=mpv(2), in1=crv(5))
                nc.vector.tensor_add(out=Y, in0=Y, in1=t0)
                rn = wt[:, 3 * DP:4 * DP]
                nc.scalar.activation(rn, q2, AF.Sqrt, bias=b_eps)
                y = wt[:, 4 * DP:5 * DP]
                nc.vector.reciprocal(out=rn, in_=rn)
                nc.vector.tensor_mul(out=y, in0=Y, in1=rn)
                hx = wt[:, 5 * DP:6 * DP]
                hy = wt[:, 6 * DP:7 * DP]
                nc.scalar.activation(hx, X, AF.Square)
                nc.scalar.activation(hy, y, AF.Square)
                h = wt[:, 7 * DP:8 * DP]
                nc.vector.tensor_add(out=h, in0=hx, in1=hy)
                rh = wt[:, 5 * DP:6 * DP]              # hx dead
                nc.scalar.activation(rh, h, AF.Sqrt, bias=b_tiny)
                c = wt[:, 0:DP]                        # q2 dead
                s = wt[:, 6 * DP:7 * DP]               # hy dead
                nc.vector.reciprocal(out=rh, in_=rh)
                nc.vector.tensor_mul(out=c, in0=X, in1=rh)
                nc.vector.tensor_mul(out=s, in0=y, in1=rh)
                # Chebyshev: cos/sin of 2phi and 3phi (reuse pl slices: b1/b3
                # component planes are dead after the cross products)
                cc_ = plv(0)
                c2 = plv(1)
                s2 = plv(2)
                c3 = plv(6)
                s3 = plv(7)
                sc = plv(8)
                nc.scalar.activation(cc_, c, AF.Square)
                nc.vector.tensor_scalar(
                    out=c2, in0=cc_, scalar1=2.0, scalar2=-1.0, op0=Op.mult, op1=Op.add)
                nc.vector.tensor_mul(out=sc, in0=s, in1=c)
                nc.vector.tensor_scalar(
                    out=s2, in0=sc, scalar1=2.0, scalar2=None, op0=Op.mult)
                nc.vector.tensor_scalar(
                    out=t0, in0=cc_, scalar1=4.0, scalar2=-3.0, op0=Op.mult, op1=Op.add)
                nc.vector.tensor_mul(out=c3, in0=t0, in1=c)
                nc.vector.tensor_scalar(
                    out=t0, in0=cc_, scalar1=4.0, scalar2=-1.0, op0=Op.mult, op1=Op.add)
                nc.vector.tensor_mul(out=s3, in0=t0, in1=s)
                ptv = pt[:].rearrange("p (n d) -> p n d", d=4)
                wi3 = workp.tile([128, 2 * DP], I32, tag="wi2")
                m2m = wi3[:, 0:DP]
                m3m = wi3[:, DP:2 * DP]
                nc.vector.tensor_scalar(
                    out=m2m, in0=ptv[:, :, 3], scalar1=2.0, scalar2=None, op0=Op.is_equal)
                nc.vector.tensor_scalar(
                    out=m3m, in0=ptv[:, :, 3], scalar1=3.0, scalar2=None, op0=Op.is_equal)
                cn = wt[:, 3 * DP:4 * DP]              # rn dead
                sn2 = wt[:, 4 * DP:5 * DP]             # y dead
                nc.vector.select(out=cn, mask=m2m, on_true=c2, on_false=c)
                nc.vector.select(out=cn, mask=m3m, on_true=c3, on_false=cn)
                nc.vector.select(out=sn2, mask=m2m, on_true=s2, on_false=s)
                nc.vector.select(out=sn2, mask=m3m, on_true=s3, on_false=sn2)
                tt1 = wt[:, 5 * DP:6 * DP]             # rh dead
                tt2 = wt[:, 6 * DP:7 * DP]             # s dead (selects done)
                nc.vector.tensor_mul(out=tt1, in0=cn, in1=ptv[:, :, 1])
                nc.vector.tensor_mul(out=tt2, in0=sn2, in1=ptv[:, :, 2])
                esum = wt[:, 7 * DP:8 * DP]            # h dead
                nc.vector.tensor_add(out=esum, in0=tt1, in1=tt2)
                nc.vector.tensor_scalar(
                    out=esum, in0=esum, scalar1=1.0, scalar2=None, op0=Op.add)
                kmt = wt[:, 0:DP]                      # c dead
                nc.vector.tensor_tensor(out=kmt, in0=ptv[:, :, 0], in1=mT, op=Op.mult)
                nc.vector.tensor_mul(out=scr, in0=esum, in1=kmt)
                nc.vector.tensor_reduce(out=rtmp, in_=scr, axis=AX.X, op=Op.add)
                nc.vector.tensor_add(out=acc[:, 2:3], in0=acc[:, 2:3], in1=rtmp)

            # ------------- final reduction: [128, 6] -> [8, 6] -> out ------
            pacc = psump.tile([8, 6], F32, tag="pacc")
            nc.tensor.matmul(out=pacc[:], lhsT=blk, rhs=acc6, start=True, stop=True)
            nc.vector.tensor_copy(out=otmp, in_=pacc[:])
            nc.vector.tensor_mul(out=otmp, in0=otmp, in1=opt6)
            nc.sync.dma_start(out=out_d.ap()[0:8, :], in_=otmp[:, 0:3])
            nc.sync.dma_start(out=out_d.ap()[8:16, :], in_=otmp[:, 3:6])

    nc.compile()
    return nc


@functools.lru_cache(maxsize=1)
def _get_nc():
    return build_nc()


def _const_row():
    """[FLATPAD] f32: per-partition constant block, see C_* layout."""
    row = np.zeros((128, LPP), np.float32)
    u = np.arange(DP)
    tail16 = np.zeros((128, 1), np.float32)
    tail16[15::16] = 1.0
    row[:, C_MT:C_MT + DP] = 1.0 - tail16 * (u >= DP - 3)[None, :]
    row[:, C_SHB:C_SHB + DP] = 999.0 * tail16 * (u >= DP - 1)[None, :]
    row[:, C_SHA:C_SHA + DP] = 999.0 * tail16 * (u >= DP - 2)[None, :]
    p = np.arange(128)
    row[:, C_BLK:C_BLK + 8] = (p[:, None] // 16 == np.arange(8)[None, :])
    row[:, C_TV:C_TV + 25] = np.arange(25)[None, :]
    return row


def make_in_maps(inputs):
    """Shard full inputs into 8 per-core single-tensor input maps."""
    feats = np.ascontiguousarray(inputs["features"], dtype=np.float32)
    Bf = feats.shape[0]
    flat = feats.reshape(Bf, -1)
    flat = np.concatenate(
        [flat, np.zeros((Bf, FLATPAD - flat.shape[1]), np.float32)], axis=1
    )
    crow = _const_row()
    crow[:, C_BT:C_BT + 30] = np.asarray(inputs["bond_type"], np.float32).reshape(-1)
    crow[:, C_AT:C_AT + 26] = np.asarray(inputs["angle_type"], np.float32).reshape(-1)
    crow[:, C_TR:C_TR + 50] = np.asarray(inputs["tor_type"], np.float32).reshape(-1)
    crow[:, C_MU:C_MU + 25] = np.asarray(inputs["multiplicity"], np.float32)
    crow[:, C_OP:C_OP + 3] = np.asarray(inputs["opt_pars"], np.float32)[0:3]
    crow_flat = crow.reshape(1, FLATPAD)
    n_nc = Bf // NS
    in_maps = []
    for k in range(n_nc):
        blob = np.concatenate([flat[NS * k:NS * (k + 1)], crow_flat], axis=0)
        in_maps.append({"features": blob})
    return in_maps


def kernel(**inputs) -> np.ndarray:
    from concourse.bass_utils import run_bass_kernel_spmd

    nc = _get_nc()
    in_maps = make_in_maps(inputs)
    res = run_bass_kernel_spmd(nc, in_maps, core_ids=list(range(len(in_maps))))
    outs = [res.results[k]["out"] for k in range(len(in_maps))]
    return np.concatenate(outs, axis=0).astype(np.float32)


def simulate_one_core(inputs, nc=None):
    """CoreSim a single NC on the first 16 samples (for correctness dev)."""
    import concourse.bass_interp as bass_interp

    if nc is None:
        nc = _get_nc()
    in_map = make_in_maps(inputs)[0]
    sim = bass_interp.MultiCoreSim(nc, 1)
    for name, val in in_map.items():
        sim.cores[0].tensor(name)[:] = val
    sim.simulate(check_with_hw=False)
    return np.array(sim.cores[0].mem_tensor("out"))


if __name__ == "__main__":
    nc = build_nc()
    print("build ok")


# revision 19
# speedup vs baseline: 1.0482x; 1.0482x over previous
"""Trainium2 Bass kernel for nn_LocalEnergyOpt (molecular-mechanics local energy).

Per batch sample (B=128): features[:, :, 5] packs coords [4096, 3]; col 6 bonds
(i,j,t)x4095; col 7 angles (i,j,k,t)x4094; col 8 torsions (i,j,k,l,t)x4093.
  e_bond = opt[0] * sum k_t (|ci-cj| - r0_t)^2
  e_ang  = opt[1] * sum k_t (theta - th0_t)^2, theta = arccos(clip(cos))
  e_tor  = opt[2] * sum k_t (1 + cos(n_t phi - d_t)), phi = atan2(y, x)
Output [B, 3].

Sharding: pure data parallel, 16 samples per NeuronCore across 8 cores.

All inputs are packed host-side into ONE dram tensor per core (per-execute
dispatch cost scales with input-buffer count): rows 0..15 are the flat
per-sample features (padded to 128*1449), row 16 is a constant block holding
the replicated param tables, tail masks, reduction selector and opt_pars.

Device pipeline per NC (2 waves x 8 samples; GPSIMD Q7 core c handles sample
8w+c on partitions 16c..16c+15):
  stage features flat -> extract packed columns (stride-9 DVE copies) ->
  dense per-sample DRAM scratch -> read back as (a) a per-partition-replicated
  coords table for ap_gather, (b) dense [128, X] index blocks -> int16
  wrap-layout index lists -> ap_gather endpoint coords -> dedup the
  16x-replicated gather outputs via a DRAM round trip -> dense [128, 256]-col
  DVE/ACT energy pipeline.

Per-edge type params for bonds/angles are NOT gathered (ap_gather costs
~15ns/idx on the Q7s and is the kernel bottleneck); instead per-type masked
sums on DVE use sum k_t (x - x0_t)^2 = k_t (S2 - 2 x0_t S1 + x0_t^2 S0) with
S0/S1/S2 accumulated per type via tensor_tensor_reduce. Torsions keep a d=4
param gather (k, cos d, sin d, n) - 25 types x 7 masked reductions would cost
more DVE than the gather costs GPSIMD.

Torsion angle avoids arccos/atan2 LUTs: cos(phi), sin(phi) are formed by
normalizing (x, y) = (n1.n2, (n1 x b2).n2 / |b2|), and cos(n phi - d) expands
via Chebyshev doubling/tripling + per-type (cos d, sin d) tables.
"""

import os
import sys
import functools

import numpy as np

ABL_NO_GATHER = bool(int(os.environ.get("ABL_NO_GATHER", "0")))
ABL_NO_ENERGY = bool(int(os.environ.get("ABL_NO_ENERGY", "0")))
ABL_NO_TYPES = bool(int(os.environ.get("ABL_NO_TYPES", "0")))

sys.path.insert(0, "/opt/trn_rl_repo")

from concourse import bacc, mybir  # noqa: E402
import concourse.tile as tile  # noqa: E402
from concourse.alu_op_type import AluOpType as Op  # noqa: E402

F32 = mybir.dt.float32
I16 = mybir.dt.int16
I32 = mybir.dt.int32
AF = mybir.ActivationFunctionType
AX = mybir.AxisListType

# Problem constants
N_CORES = 8
NS = 16                      # samples per NeuronCore
NB, NA, NT = 4095, 4094, 4093
NATOMS = 4096
MAXLEN = 20465
# per-sample packed-column row: [coords 12288 | bonds 12288 | angles 16384
# | torsions 20480] (host pre-slices columns 5..8 of features; the other 5
# columns are never read by the energy computation)
OFF5, OFF6, OFF7, OFF8 = 0, 12288, 24576, 40960
ROWLEN = 61440
CPP = 960                    # const-block floats per partition (2 rows)
EPS = 1e-8
PI = float(np.pi)

LIST = 4096                  # per-core index list length per class (padded)
GCH = 1024                   # ap_gather chunk
DP = LIST // 16              # 256 dense positions per partition

# const-row per-partition float layout
C_MT = 0                     # [256] torsion valid mask (1 valid / 0 tail)
C_SHB = 256                  # [256] bond type tail shift (0 / 999)
C_SHA = 512                  # [256] angle type tail shift (0 / 999)
C_BT = 768                   # [30] bond_type (k, r0) x 15
C_AT = 798                   # [26] angle_type (k, th0) x 13
C_TR = 824                   # [50] tor_type (k, delta) x 25
C_MU = 874                   # [25] multiplicity (f32)
C_OP = 899                   # [3] opt_pars[0:3]
C_BLK = 902                  # [8] PE group selector row (p//16 == c)
C_TV = 910                   # [25] type values 0..24 (for batched is_equal)
C_END = 935


def build_nc():
    nc = bacc.Bacc(None, target_bir_lowering=False, debug=False)

    feat = nc.dram_tensor("features", [NS + 2, ROWLEN], F32, kind="ExternalInput")
    out_d = nc.dram_tensor("out", [NS, 3], F32, kind="ExternalOutput")

    with tile.TileContext(nc) as tc:
        with (
            tc.tile_pool(name="table", bufs=1) as tablep,
            tc.tile_pool(name="idx16", bufs=2) as idx16p,
            tc.tile_pool(name="gath", bufs=2) as gathp,
            tc.tile_pool(name="const", bufs=1) as constp,
            tc.tile_pool(name="idxraw", bufs=2) as idxrawp,
            tc.tile_pool(name="dense", bufs=4) as densep,
            tc.tile_pool(name="tcol", bufs=2) as tcolp,
            tc.tile_pool(name="work", bufs=1) as workp,
            tc.tile_pool(name="accp", bufs=1) as accp,
            tc.tile_pool(name="psum", bufs=1, space="PSUM") as psump,
            tc.tile_pool(name="dramded", bufs=6, space="DRAM") as dedp,
        ):
            # ---------------- constants ------------------------------------
            cst = constp.tile([128, C_END], F32)
            nc.sync.dma_start(
                out=cst[:],
                in_=feat.ap()[NS:NS + 2].rearrange("a f -> (a f)").rearrange(
                    "(p f) -> p f", f=CPP)[:, 0:C_END],
            )
            mT = cst[:, C_MT:C_MT + DP]
            shB = cst[:, C_SHB:C_SHB + DP]
            shA = cst[:, C_SHA:C_SHA + DP]
            btab = cst[:, C_BT:C_BT + 30]
            atab = cst[:, C_AT:C_AT + 26]
            traw = cst[:, C_TR:C_TR + 50]
            multf = cst[:, C_MU:C_MU + 25]
            opt3 = cst[:, C_OP:C_OP + 3]
            blk = cst[:, C_BLK:C_BLK + 8]
            tv25 = cst[:, C_TV:C_TV + 25]

            fwork = constp.tile([128, 136], F32)
            cb = fwork[:, 0:4]
            nc.vector.memset(cb[:, 0:1], EPS)
            nc.vector.memset(cb[:, 1:2], PI / 2.0)
            nc.vector.memset(cb[:, 2:3], 1e-30)
            nc.vector.memset(cb[:, 3:4], -1.0)
            b_eps = cb[:, 0:1]
            b_pi2 = cb[:, 1:2]
            b_tiny = cb[:, 2:3]
            s_neg1 = cb[:, 3:4]

            # torsion derived table (k, cos d, sin d, n) x 25, on all parts
            ttab = fwork[:, 4:104]
            t4v = ttab.rearrange("p (n d) -> p n d", d=4)
            trv = traw.rearrange("p (n d) -> p n d", d=2)
            nc.vector.tensor_copy(out=t4v[:, :, 0], in_=trv[:, :, 0])          # k
            # cos d = sin(pi/2 - d); d in [0, 3.15) keeps the arg in [-pi, pi]
            carg = fwork[:, 110:135]
            nc.vector.tensor_scalar(out=carg, in0=trv[:, :, 1], scalar1=-1.0,
                                    scalar2=PI / 2.0, op0=Op.mult, op1=Op.add)
            nc.scalar.activation(t4v[:, :, 1], carg, AF.Sin)
            nc.scalar.activation(t4v[:, :, 2], trv[:, :, 1], AF.Sin)           # sin d
            nc.vector.tensor_copy(out=t4v[:, :, 3], in_=multf)                 # n

            opt6 = fwork[0:8, 104:110]
            nc.vector.tensor_copy(out=opt6[:, 0:3], in_=opt3[0:8, :])
            nc.vector.tensor_copy(out=opt6[:, 3:6], in_=opt3[0:8, :])

            accb = accp.tile([128, DP + 6 + 8], F32)
            scr = accb[:, 0:DP]            # TTR mandatory elementwise out
            acc6 = accb[:, DP:DP + 6]
            otmp = accb[0:8, DP + 6:DP + 12]
            rtmp = accb[:, DP + 12:DP + 13]

            btv = btab.rearrange("p (n d) -> p n d", d=2)
            atv = atab.rearrange("p (n d) -> p n d", d=2)
            ttab4 = ttab.rearrange("p (n d) -> p n d", d=4)

            # ========== PREP PHASE: both waves' staging + index prep =====
            # (emitted before any energy math so wave-1 gather prerequisites
            # are not queued behind wave-0 energy on the DVE)
            wave_state = []
            for w in range(2):
                iraw = idxrawp.tile([128, 3072], F32, tag="iraw")
                braw = iraw[:, 0:768]
                araw = iraw[:, 768:1792]
                trawi = iraw[:, 1792:3072]
                rows = feat.ap()[8 * w:8 * w + 8]
                nc.sync.dma_start(
                    out=braw,
                    in_=rows[:, OFF6:OFF6 + 12288].rearrange(
                        "a (j f) -> a j f", j=16),
                )
                nc.sync.dma_start(
                    out=araw,
                    in_=rows[:, OFF7:OFF7 + 16384].rearrange(
                        "a (j f) -> a j f", j=16),
                )
                nc.sync.dma_start(
                    out=trawi,
                    in_=rows[:, OFF8:OFF8 + 20480].rearrange(
                        "a (j f) -> a j f", j=16),
                )

                idxt = idx16p.tile([128, 10 * 256], I16, tag="idxt")
                bv = braw.rearrange("p (e k) -> p e k", k=3)
                av = araw.rearrange("p (e k) -> p e k", k=4)
                tv = trawi.rearrange("p (e k) -> p e k", k=5)
                for k in range(2):
                    nc.vector.tensor_copy(out=idxt[:, 256 * k:256 * (k + 1)],
                                          in_=bv[:, :, k])
                for k in range(3):
                    nc.vector.tensor_copy(out=idxt[:, 256 * (2 + k):256 * (3 + k)],
                                          in_=av[:, :, k])
                for k in range(5):
                    nc.vector.tensor_copy(out=idxt[:, 256 * (5 + k):256 * (6 + k)],
                                          in_=tv[:, :, k])
                # lists: 0,1 = bond i,j; 2..4 = angle i,j,k; 5..9 = tor i,j,k,l,t

                tcols = tcolp.tile([128, 2 * DP], F32, tag="tcols")
                tB = tcols[:, 0:DP]
                tA = tcols[:, DP:2 * DP]
                nc.vector.tensor_tensor(out=tB, in0=bv[:, :, 2], in1=shB, op=Op.add)
                nc.vector.tensor_tensor(out=tA, in0=av[:, :, 3], in1=shA, op=Op.add)
                wave_state.append((idxt, tB, tA))

            # ========== COMPUTE PHASE: per-wave gathers + energy ==========
            for w in range(2):
                idxt, tB, tA = wave_state[w]

                def idx_list(n):
                    return idxt[:, 256 * n:256 * (n + 1)]

                # gather table (replicated coords): partition p holds sample
                # (p//16)'s coords; step-0 src AP replicates each sample's
                # row across its 16 partitions
                table = tablep.tile([128, 3 * NATOMS], F32, tag="table")
                nc.sync.dma_start(
                    out=table[:],
                    in_=feat.ap()[8 * w:8 * w + 8][:, None, OFF5:OFF5 + 3 * NATOMS]
                        .to_broadcast([8, 16, 3 * NATOMS]),
                )

                def gather_dedup(idx_n, tab_ap, n_elems, d):
                    """ap_gather LIST indices; dedup 16x replicas via DRAM.
                    Returns dense [128, DP*d] tile (edge position 256j+u on
                    partition 16c+j at cols u*d..). Chunk size keeps every
                    call inside the same 24KB/partition pool slot."""
                    gch = 2048 if d == 3 else 1024
                    ded = dedp.tile([8, LIST * d], F32, tag="ded")
                    for ch in range(LIST // gch):
                        g = gathp.tile([128, gch * d], F32, tag="g", name="g")
                        if ABL_NO_GATHER:
                            nc.vector.memset(g[0:8, 0:1], 0.0)
                        else:
                            nc.gpsimd.ap_gather(
                                out_ap=g[:].rearrange("p (n d) -> p n d", d=d),
                                in_ap=tab_ap,
                                idxs_ap=idx_list(idx_n)[:, (gch // 16) * ch:(gch // 16) * (ch + 1)],
                                channels=128,
                                num_elems=n_elems,
                                d=d,
                                num_idxs=gch,
                            )
                        nc.sync.dma_start(
                            out=ded[:][:, gch * d * ch:gch * d * (ch + 1)],
                            in_=g[:].rearrange("(a b) f -> a b f", b=16)[:, 0, :],
                        )
                    dn = densep.tile([128, DP * d], F32, tag="dense")
                    ded_flat = ded[:].rearrange("s f -> (s f)")
                    nc.sync.dma_start(
                        out=dn[:],
                        in_=ded_flat.rearrange("(p f) -> p f", f=DP * d),
                    )
                    return dn

                tab3 = table[:].rearrange("p (n d) -> p n d", d=3)

                acc = acc6[:, 3 * w:3 * w + 3]
                nc.vector.memset(acc, 0.0)

                def type_masked_energy(xcol, tcol, ptab, n_types, acc_col):
                    """acc_col += sum_t k_t sum_{e: t(e)=t} (x_e - x0_t)^2,
                    processed 4 types per batch with broadcast APs."""
                    if ABL_NO_TYPES:
                        nc.vector.tensor_reduce(out=rtmp, in_=xcol, axis=AX.X, op=Op.add)
                        nc.vector.tensor_add(out=acc_col, in0=acc_col, in1=rtmp)
                        return
                    st = workp.tile([128, 32], F32, tag="wst")
                    S = st[:, 0:n_types]
                    wide = workp.tile([128, 9 * DP], F32, tag="w9")
                    for t0 in range(0, n_types, 4):
                        nb = min(4, n_types - t0)
                        msk = wide[:, 0:nb * DP].rearrange(
                            "p (t e) -> p t e", e=DP)
                        dif = wide[:, 4 * DP:(4 + nb) * DP].rearrange(
                            "p (t e) -> p t e", e=DP)
                        nc.vector.tensor_tensor(
                            out=msk,
                            in0=tcol[:, None, :].to_broadcast([128, nb, DP]),
                            in1=tv25[:, t0:t0 + nb][:, :, None].to_broadcast(
                                [128, nb, DP]),
                            op=Op.is_equal)
                        nc.vector.tensor_tensor(
                            out=dif,
                            in0=xcol[:, None, :].to_broadcast([128, nb, DP]),
                            in1=ptab[:, t0:t0 + nb, 1][:, :, None].to_broadcast(
                                [128, nb, DP]),
                            op=Op.subtract)
                        nc.scalar.activation(dif, dif, AF.Square)
                        nc.vector.tensor_tensor(out=msk, in0=msk, in1=dif,
                                                op=Op.mult)
                        nc.vector.tensor_reduce(
                            out=S[:, t0:t0 + nb], in_=msk, axis=AX.X, op=Op.add)
                    cmb = workp.tile([128, 32], F32, tag="wcmb")
                    c1 = cmb[:, 0:n_types]
                    nc.vector.tensor_tensor(out=c1, in0=S, in1=ptab[:, 0:n_types, 0],
                                            op=Op.mult)
                    nc.vector.tensor_reduce(out=rtmp, in_=c1, axis=AX.X, op=Op.add)
                    nc.vector.tensor_add(out=acc_col, in0=acc_col, in1=rtmp)

                # ==================== BONDS ====================
                ci = gather_dedup(0, tab3, NATOMS, 3)
                cj = gather_dedup(1, tab3, NATOMS, 3)
                d3 = workp.tile([128, 3 * DP], F32, tag="w3a")
                nc.vector.tensor_sub(out=d3[:], in0=ci[:], in1=cj[:])
                d3s = workp.tile([128, 3 * DP], F32, tag="w3b")
                nc.vector.tensor_mul(out=d3s[:], in0=d3[:], in1=d3[:])
                wb = workp.tile([128, 8 * DP], F32, tag="w8")
                r2 = wb[:, 0:DP]
                nc.vector.tensor_reduce(
                    out=r2, in_=d3s[:].rearrange("p (n d) -> p n d", d=3),
                    axis=AX.X, op=Op.add,
                )
                r = wb[:, DP:2 * DP]
                nc.scalar.activation(r, r2, AF.Sqrt, bias=b_eps)
                type_masked_energy(r, tB, btv, 15, acc[:, 0:1])

                # ==================== ANGLES ====================
                gi = gather_dedup(2, tab3, NATOMS, 3)
                gj = gather_dedup(3, tab3, NATOMS, 3)
                gk = gather_dedup(4, tab3, NATOMS, 3)
                v1 = workp.tile([128, 3 * DP], F32, tag="w3a")
                v2 = workp.tile([128, 3 * DP], F32, tag="w3b")
                nc.vector.tensor_sub(out=v1[:], in0=gi[:], in1=gj[:])
                nc.vector.tensor_sub(out=v2[:], in0=gk[:], in1=gj[:])
                prod = workp.tile([128, 3 * DP], F32, tag="w3c")
                wa = workp.tile([128, 8 * DP], F32, tag="w8")
                d11 = wa[:, 0:DP]
                d22 = wa[:, 1 * DP:2 * DP]
                d12 = wa[:, 2 * DP:3 * DP]

                def dot3(dst, a, b):
                    nc.vector.tensor_mul(out=prod[:], in0=a[:], in1=b[:])
                    nc.vector.tensor_reduce(
                        out=dst, in_=prod[:].rearrange("p (n d) -> p n d", d=3),
                        axis=AX.X, op=Op.add,
                    )

                dot3(d11, v1, v1)
                dot3(d22, v2, v2)
                dot3(d12, v1, v2)
                s1 = wa[:, 3 * DP:4 * DP]
                s2a = wa[:, 4 * DP:5 * DP]
                nc.scalar.activation(s1, d11, AF.Sqrt, bias=b_eps)
                nc.scalar.activation(s2a, d22, AF.Sqrt, bias=b_eps)
                den = wa[:, 5 * DP:6 * DP]
                nc.vector.tensor_mul(out=den, in0=s1, in1=s2a)
                cosv = wa[:, 6 * DP:7 * DP]
                nc.vector.reciprocal(out=den, in_=den)
                nc.vector.tensor_mul(out=cosv, in0=d12, in1=den)
                cosc = wa[:, 7 * DP:8 * DP]
                nc.vector.tensor_scalar(
                    out=cosc, in0=cosv, scalar1=-1.0 + 1e-6, scalar2=1.0 - 1e-6,
                    op0=Op.max, op1=Op.min,
                )
                # theta = arccos(cosc) via two bounded-arg arctan branches
                # (ACT Arctan domain is [-pi/2, pi/2] so |arg| <= 1 required):
                #  |c| >  s: theta = arctan(s/c) + pi*(c<0)
                #  |c| <= s: theta = pi/2 - arctan(c/s), s = sqrt(1-c^2)
                cc = wa[:, 0:DP]                       # d11 dead
                nc.scalar.activation(cc, cosc, AF.Square)
                om = wa[:, 1 * DP:2 * DP]              # d22 dead
                nc.vector.tensor_scalar(
                    out=om, in0=cc, scalar1=-1.0, scalar2=1.0, op0=Op.mult, op1=Op.add
                )
                sn = wa[:, 2 * DP:3 * DP]              # d12 dead
                nc.scalar.activation(sn, om, AF.Sqrt)
                sgn = wa[:, 3 * DP:4 * DP]             # s1 dead
                nc.vector.tensor_scalar(
                    out=sgn, in0=cosc, scalar1=0.0, scalar2=None, op0=Op.is_ge)
                nc.vector.tensor_scalar(
                    out=sgn, in0=sgn, scalar1=2e-18, scalar2=-1e-18,
                    op0=Op.mult, op1=Op.add)
                csafe = wa[:, 4 * DP:5 * DP]           # s2a dead
                nc.vector.tensor_add(out=csafe, in0=cosc, in1=sgn)
                ra = wa[:, 3 * DP:4 * DP]              # sgn dead
                nc.vector.reciprocal(out=csafe, in_=csafe)
                nc.vector.tensor_mul(out=ra, in0=sn, in1=csafe)
                nc.vector.tensor_scalar(
                    out=ra, in0=ra, scalar1=-1.0, scalar2=1.0, op0=Op.max, op1=Op.min)
                ata = wa[:, 4 * DP:5 * DP]             # csafe dead
                nc.scalar.activation(ata, ra, AF.Arctan)
                corr = wa[:, 5 * DP:6 * DP]            # den dead
                nc.vector.tensor_scalar(
                    out=corr, in0=cosc, scalar1=0.0, scalar2=PI, op0=Op.is_lt, op1=Op.mult
                )
                tha = wa[:, 3 * DP:4 * DP]             # ra dead
                nc.vector.tensor_add(out=tha, in0=ata, in1=corr)
                rb = wa[:, 4 * DP:5 * DP]              # ata dead
                nc.vector.reciprocal(out=sn, in_=sn)
                nc.vector.tensor_mul(out=rb, in0=cosc, in1=sn)
                nc.vector.tensor_scalar(
                    out=rb, in0=rb, scalar1=-1.0, scalar2=1.0, op0=Op.max, op1=Op.min)
                thb = wa[:, 5 * DP:6 * DP]             # corr dead
                nc.scalar.activation(thb, rb, AF.Arctan)
                nc.vector.tensor_scalar(
                    out=thb, in0=thb, scalar1=-1.0, scalar2=PI / 2.0,
                    op0=Op.mult, op1=Op.add)
                wi2 = workp.tile([128, DP], I32, tag="wi")
                mbr = wi2[:, 0:DP]
                nc.vector.tensor_scalar(
                    out=mbr, in0=cc, scalar1=0.5, scalar2=None, op0=Op.is_gt)
                th = wa[:, 6 * DP:7 * DP]              # cosv dead
                nc.vector.select(out=th, mask=mbr, on_true=tha, on_false=thb)
                type_masked_energy(th, tA, atv, 13, acc[:, 1:2])

                # ==================== TORSIONS ====================
                ti = gather_dedup(5, tab3, NATOMS, 3)
                tj = gather_dedup(6, tab3, NATOMS, 3)
                tk_ = gather_dedup(7, tab3, NATOMS, 3)
                tl = gather_dedup(8, tab3, NATOMS, 3)
                b1 = workp.tile([128, 3 * DP], F32, tag="w3a")
                b2 = workp.tile([128, 3 * DP], F32, tag="w3b")
                b3 = workp.tile([128, 3 * DP], F32, tag="w3c")
                nc.vector.tensor_sub(out=b1[:], in0=tj[:], in1=ti[:])
                nc.vector.tensor_sub(out=b2[:], in0=tk_[:], in1=tj[:])
                nc.vector.tensor_sub(out=b3[:], in0=tl[:], in1=tk_[:])
                pt = gather_dedup(9, ttab4, 25, 4)
                pl = workp.tile([128, 9 * DP], F32, tag="w9")

                def plv(n):
                    return pl[:, DP * n:DP * (n + 1)]

                for m in range(3):
                    nc.vector.tensor_copy(
                        out=plv(0 + m),
                        in_=b1[:].rearrange("p (n d) -> p n d", d=3)[:, :, m])
                    nc.vector.tensor_copy(
                        out=plv(3 + m),
                        in_=b2[:].rearrange("p (n d) -> p n d", d=3)[:, :, m])
                    nc.vector.tensor_copy(
                        out=plv(6 + m),
                        in_=b3[:].rearrange("p (n d) -> p n d", d=3)[:, :, m])
                # n1 = b1 x b2 -> cr 0..2 ; n2 = b2 x b3 -> cr 3..5
                cr_ = workp.tile([128, 6 * DP], F32, tag="w6")

                def crv(n):
                    return cr_[:, DP * n:DP * (n + 1)]

                tmp = workp.tile([128, 2 * DP], F32, tag="w2")
                t0 = tmp[:, 0:DP]
                t1_ = tmp[:, DP:2 * DP]
                for m in range(3):
                    mp1, mp2 = (m + 1) % 3, (m + 2) % 3
                    nc.vector.tensor_mul(out=t0, in0=plv(0 + mp1), in1=plv(3 + mp2))
                    nc.vector.tensor_mul(out=t1_, in0=plv(0 + mp2), in1=plv(3 + mp1))
                    nc.vector.tensor_sub(out=crv(m), in0=t0, in1=t1_)
                    nc.vector.tensor_mul(out=t0, in0=plv(3 + mp1), in1=plv(6 + mp2))
                    nc.vector.tensor_mul(out=t1_, in0=plv(3 + mp2), in1=plv(6 + mp1))
                    nc.vector.tensor_sub(out=crv(3 + m), in0=t0, in1=t1_)
                wt = workp.tile([128, 8 * DP], F32, tag="w8")
                q2 = wt[:, 0:DP]
                nc.vector.tensor_mul(out=b1[:], in0=b2[:], in1=b2[:])  # b1 = scratch
                nc.vector.tensor_reduce(
                    out=q2, in_=b1[:].rearrange("p (n d) -> p n d", d=3),
                    axis=AX.X, op=Op.add,
                )
                # m1' = n1 x b2 (normalization folded into rn)
                mp = workp.tile([128, 3 * DP], F32, tag="w3a")

                def mpv(n):
                    return mp[:, DP * n:DP * (n + 1)]

                for m in range(3):
                    mp1, mp2 = (m + 1) % 3, (m + 2) % 3
                    nc.vector.tensor_mul(out=t0, in0=crv(mp1), in1=plv(3 + mp2))
                    nc.vector.tensor_mul(out=t1_, in0=crv(mp2), in1=plv(3 + mp1))
                    nc.vector.tensor_sub(out=mpv(m), in0=t0, in1=t1_)
                X = wt[:, 1 * DP:2 * DP]
                Y = wt[:, 2 * DP:3 * DP]
                nc.vector.tensor_mul(out=t0, in0=crv(0), in1=crv(3))
                nc.vector.tensor_mul(out=t1_, in0=crv(1), in1=crv(4))
                nc.vector.tensor_add(out=X, in0=t0, in1=t1_)
                nc.vector.tensor_mul(out=t0, in0=crv(2), in1=crv(5))
                nc.vector.tensor_add(out=X, in0=X, in1=t0)
                nc.vector.tensor_mul(out=t0, in0=mpv(0), in1=crv(3))
                nc.vector.tensor_mul(out=t1_, in0=mpv(1), in1=crv(4))
                nc.vector.tensor_add(out=Y, in0=t0, in1=t1_)
                nc.vector.tensor_mul(out=t0, in0=mpv(2), in1=crv(5))
                nc.vector.tensor_add(out=Y, in0=Y, in1=t0)
                rn = wt[:, 3 * DP:4 * DP]
                nc.scalar.activation(rn, q2, AF.Sqrt, bias=b_eps)
                y = wt[:, 4 * DP:5 * DP]
                nc.vector.reciprocal(out=rn, in_=rn)
                nc.vector.tensor_mul(out=y, in0=Y, in1=rn)
                hx = wt[:, 5 * DP:6 * DP]
                hy = wt[:, 6 * DP:7 * DP]
                nc.scalar.activation(hx, X, AF.Square)
                nc.scalar.activation(hy, y, AF.Square)
                h = wt[:, 7 * DP:8 * DP]
                nc.vector.tensor_add(out=h, in0=hx, in1=hy)
                rh = wt[:, 5 * DP:6 * DP]              # hx dead
                nc.scalar.activation(rh, h, AF.Sqrt, bias=b_tiny)
                c = wt[:, 0:DP]                        # q2 dead
                s = wt[:, 6 * DP:7 * DP]               # hy dead
                nc.vector.reciprocal(out=rh, in_=rh)
                nc.vector.tensor_mul(out=c, in0=X, in1=rh)
                nc.vector.tensor_mul(out=s, in0=y, in1=rh)
                # Chebyshev: cos/sin of 2phi and 3phi (reuse pl slices: b1/b3
                # component planes are dead after the cross products)
                cc_ = plv(0)
                c2 = plv(1)
                s2 = plv(2)
                c3 = plv(6)
                s3 = plv(7)
                sc = plv(8)
                nc.scalar.activation(cc_, c, AF.Square)
                nc.vector.tensor_scalar(
                    out=c2, in0=cc_, scalar1=2.0, scalar2=-1.0, op0=Op.mult, op1=Op.add)
                nc.vector.tensor_mul(out=sc, in0=s, in1=c)
                nc.vector.tensor_scalar(
                    out=s2, in0=sc, scalar1=2.0, scalar2=None, op0=Op.mult)
                nc.vector.tensor_scalar(
                    out=t0, in0=cc_, scalar1=4.0, scalar2=-3.0, op0=Op.mult, op1=Op.add)
                nc.vector.tensor_mul(out=c3, in0=t0, in1=c)
                nc.vector.tensor_scalar(
                    out=t0, in0=cc_, scalar1=4.0, scalar2=-1.0, op0=Op.mult, op1=Op.add)
                nc.vector.tensor_mul(out=s3, in0=t0, in1=s)
                ptv = pt[:].rearrange("p (n d) -> p n d", d=4)
                wi3 = workp.tile([128, 2 * DP], I32, tag="wi2")
                m2m = wi3[:, 0:DP]
                m3m = wi3[:, DP:2 * DP]
                nc.vector.tensor_scalar(
                    out=m2m, in0=ptv[:, :, 3], scalar1=2.0, scalar2=None, op0=Op.is_equal)
                nc.vector.tensor_scalar(
                    out=m3m, in0=ptv[:, :, 3], scalar1=3.0, scalar2=None, op0=Op.is_equal)
                cn = wt[:, 3 * DP:4 * DP]              # rn dead
                sn2 = wt[:, 4 * DP:5 * DP]             # y dead
                nc.vector.select(out=cn, mask=m2m, on_true=c2, on_false=c)
                nc.vector.select(out=cn, mask=m3m, on_true=c3, on_false=cn)
                nc.vector.select(out=sn2, mask=m2m, on_true=s2, on_false=s)
                nc.vector.select(out=sn2, mask=m3m, on_true=s3, on_false=sn2)
                tt1 = wt[:, 5 * DP:6 * DP]             # rh dead
                tt2 = wt[:, 6 * DP:7 * DP]             # s dead (selects done)
                nc.vector.tensor_mul(out=tt1, in0=cn, in1=ptv[:, :, 1])
                nc.vector.tensor_mul(out=tt2, in0=sn2, in1=ptv[:, :, 2])
                esum = wt[:, 7 * DP:8 * DP]            # h dead
                nc.vector.tensor_add(out=esum, in0=tt1, in1=tt2)
                nc.vector.tensor_scalar(
                    out=esum, in0=esum, scalar1=1.0, scalar2=None, op0=Op.add)
                kmt = wt[:, 0:DP]                      # c dead
                nc.vector.tensor_tensor(out=kmt, in0=ptv[:, :, 0], in1=mT, op=Op.mult)
                nc.vector.tensor_mul(out=scr, in0=esum, in1=kmt)
                nc.vector.tensor_reduce(out=rtmp, in_=scr, axis=AX.X, op=Op.add)
                nc.vector.tensor_add(out=acc[:, 2:3], in0=acc[:, 2:3], in1=rtmp)

            # ------------- final reduction: [128, 6] -> [8, 6] -> out ------
            pacc = psump.tile([8, 6], F32, tag="pacc")
            nc.tensor.matmul(out=pacc[:], lhsT=blk, rhs=acc6, start=True, stop=True)
            nc.vector.tensor_copy(out=otmp, in_=pacc[:])
            nc.vector.tensor_mul(out=otmp, in0=otmp, in1=opt6)
            nc.sync.dma_start(out=out_d.ap()[0:8, :], in_=otmp[:, 0:3])
            nc.sync.dma_start(out=out_d.ap()[8:16, :], in_=otmp[:, 3:6])

    nc.compile()
    return nc


@functools.lru_cache(maxsize=1)
def _get_nc():
    return build_nc()


def _const_row():
    """[128, CPP] f32: per-partition constant block, see C_* layout."""
    row = np.zeros((128, CPP), np.float32)
    u = np.arange(DP)
    tail16 = np.zeros((128, 1), np.float32)
    tail16[15::16] = 1.0
    row[:, C_MT:C_MT + DP] = 1.0 - tail16 * (u >= DP - 3)[None, :]
    row[:, C_SHB:C_SHB + DP] = 999.0 * tail16 * (u >= DP - 1)[None, :]
    row[:, C_SHA:C_SHA + DP] = 999.0 * tail16 * (u >= DP - 2)[None, :]
    p = np.arange(128)
    row[:, C_BLK:C_BLK + 8] = (p[:, None] // 16 == np.arange(8)[None, :])
    row[:, C_TV:C_TV + 25] = np.arange(25)[None, :]
    return row


def make_in_maps(inputs):
    """Shard full inputs into 8 per-core single-tensor input maps.

    Pure layout: slice the 4 used feature columns (coords/bonds/angles/
    torsions) into per-sample packed rows; append 2 rows of per-partition
    constants (param tables, tail masks, selector)."""
    feats = np.asarray(inputs["features"], dtype=np.float32)
    Bf = feats.shape[0]
    rows = np.concatenate([
        feats[:, 0:12288, 5],
        feats[:, 0:12288, 6],
        feats[:, 0:16384, 7],
        feats[:, 0:20465, 8],
        np.zeros((Bf, 15), np.float32),          # pad torsion col to 20480
    ], axis=1)                                   # [B, ROWLEN]
    crow = _const_row()
    crow[:, C_BT:C_BT + 30] = np.asarray(inputs["bond_type"], np.float32).reshape(-1)
    crow[:, C_AT:C_AT + 26] = np.asarray(inputs["angle_type"], np.float32).reshape(-1)
    crow[:, C_TR:C_TR + 50] = np.asarray(inputs["tor_type"], np.float32).reshape(-1)
    crow[:, C_MU:C_MU + 25] = np.asarray(inputs["multiplicity"], np.float32)
    crow[:, C_OP:C_OP + 3] = np.asarray(inputs["opt_pars"], np.float32)[0:3]
    crow_flat = crow.reshape(2, ROWLEN)
    n_nc = Bf // NS
    in_maps = []
    for k in range(n_nc):
        blob = np.concatenate([rows[NS * k:NS * (k + 1)], crow_flat], axis=0)
        in_maps.append({"features": np.ascontiguousarray(blob)})
    return in_maps


def kernel(**inputs) -> np.ndarray:
    from concourse.bass_utils import run_bass_kernel_spmd

    nc = _get_nc()
    in_maps = make_in_maps(inputs)
    res = run_bass_kernel_spmd(nc, in_maps, core_ids=list(range(len(in_maps))))
    outs = [res.results[k]["out"] for k in range(len(in_maps))]
    return np.concatenate(outs, axis=0).astype(np.float32)


def simulate_one_core(inputs, nc=None):
    """CoreSim a single NC on the first 16 samples (for correctness dev)."""
    import concourse.bass_interp as bass_interp

    if nc is None:
        nc = _get_nc()
    in_map = make_in_maps(inputs)[0]
    sim = bass_interp.MultiCoreSim(nc, 1)
    for name, val in in_map.items():
        sim.cores[0].tensor(name)[:] = val
    sim.simulate(check_with_hw=False)
    return np.array(sim.cores[0].mem_tensor("out"))


if __name__ == "__main__":
    nc = build_nc()
    print("build ok")


# revision 20
# speedup vs baseline: 1.0557x; 1.0072x over previous
"""Trainium2 Bass kernel for nn_LocalEnergyOpt (molecular-mechanics local energy).

Per batch sample (B=128): features[:, :, 5] packs coords [4096, 3]; col 6 bonds
(i,j,t)x4095; col 7 angles (i,j,k,t)x4094; col 8 torsions (i,j,k,l,t)x4093.
  e_bond = opt[0] * sum k_t (|ci-cj| - r0_t)^2
  e_ang  = opt[1] * sum k_t (theta - th0_t)^2, theta = arccos(clip(cos))
  e_tor  = opt[2] * sum k_t (1 + cos(n_t phi - d_t)), phi = atan2(y, x)
Output [B, 3].

Sharding: pure data parallel, 16 samples per NeuronCore across 8 cores.

All inputs are packed host-side into ONE dram tensor per core (per-execute
dispatch cost scales with input-buffer count). Only feature columns 5..8 are
ever read by the computation, so make_in_maps ships them pre-sliced (pure
layout, no arithmetic) as packed per-sample rows [coords|bonds|angles|
torsions]; rows 16..17 are a per-partition constant block holding the
replicated param tables, tail masks, reduction selector and opt_pars. This
cuts input bytes ~3x and removes the on-device stage/extract/column-scratch
pipeline entirely.

Device pipeline per NC (2 waves x 8 samples; GPSIMD Q7 core c handles sample
8w+c on partitions 16c..16c+15). Prep for BOTH waves is emitted before any
energy math so wave-1 gather prerequisites never queue behind wave-0 energy
ops (ap_gather stalls every other engine while the Q7s run, so the kernel
minimizes total engine-time rather than chasing overlap):
  DMA index blocks straight from the input rows -> int16 wrap-layout index
  lists -> per-wave: DMA-broadcast the coords table (each sample replicated
  across its 16 partitions) -> ap_gather endpoint coords -> dedup the
  16x-replicated gather outputs via a DRAM round trip -> dense [128, 256]-col
  DVE/ACT energy pipeline.

Per-edge type params for bonds/angles are NOT gathered (ap_gather costs
~25-30ns/idx in-kernel and is the bottleneck); instead per-type masked sums
sum k_t (x_e - x0_t)^2 run on DVE, batched 4 types at a time with broadcast
(stride-0) access patterns. Torsions keep a d=4 param gather
(k, cos d, sin d, n) - 25 types x 7 masked reductions would cost more DVE
than the gather costs GPSIMD.

Torsion angle avoids arccos/atan2 LUTs: cos(phi), sin(phi) are formed by
normalizing (x, y) = (n1.n2, (n1 x b2).n2 / |b2|), and cos(n phi - d) expands
via Chebyshev doubling/tripling + per-type (cos d, sin d) tables.
"""

import os
import sys
import functools

import numpy as np

ABL_NO_GATHER = bool(int(os.environ.get("ABL_NO_GATHER", "0")))
ABL_NO_ENERGY = bool(int(os.environ.get("ABL_NO_ENERGY", "0")))
ABL_NO_TYPES = bool(int(os.environ.get("ABL_NO_TYPES", "0")))

sys.path.insert(0, "/opt/trn_rl_repo")

from concourse import bacc, mybir  # noqa: E402
import concourse.tile as tile  # noqa: E402
from concourse.alu_op_type import AluOpType as Op  # noqa: E402

F32 = mybir.dt.float32
I16 = mybir.dt.int16
I32 = mybir.dt.int32
AF = mybir.ActivationFunctionType
AX = mybir.AxisListType

# Problem constants
N_CORES = 8
NS = 16                      # samples per NeuronCore
NB, NA, NT = 4095, 4094, 4093
NATOMS = 4096
MAXLEN = 20465
# per-sample packed-column row: [coords 12288 | bonds 12288 | angles 16384
# | torsions 20480] (host pre-slices columns 5..8 of features; the other 5
# columns are never read by the energy computation)
OFF5, OFF6, OFF7, OFF8 = 0, 12288, 24576, 40960
ROWLEN = 61440
CPP = 960                    # const-block floats per partition (2 rows)
EPS = 1e-8
PI = float(np.pi)

LIST = 4096                  # per-core index list length per class (padded)
GCH = 1024                   # ap_gather chunk
DP = LIST // 16              # 256 dense positions per partition

# const-row per-partition float layout
C_MT = 0                     # [256] torsion valid mask (1 valid / 0 tail)
C_SHB = 256                  # [256] bond type tail shift (0 / 999)
C_SHA = 512                  # [256] angle type tail shift (0 / 999)
C_BT = 768                   # [30] bond_type (k, r0) x 15
C_AT = 798                   # [26] angle_type (k, th0) x 13
C_TR = 824                   # [50] tor_type (k, delta) x 25
C_MU = 874                   # [25] multiplicity (f32)
C_OP = 899                   # [3] opt_pars[0:3]
C_BLK = 902                  # [8] PE group selector row (p//16 == c)
C_TV = 910                   # [25] type values 0..24 (for batched is_equal)
C_END = 935


def build_nc():
    nc = bacc.Bacc(None, target_bir_lowering=False, debug=False)

    feat = nc.dram_tensor("features", [NS + 2, ROWLEN], F32, kind="ExternalInput")
    out_d = nc.dram_tensor("out", [NS, 3], F32, kind="ExternalOutput")

    with tile.TileContext(nc) as tc:
        with (
            tc.tile_pool(name="table", bufs=1) as tablep,
            tc.tile_pool(name="idx16", bufs=2) as idx16p,
            tc.tile_pool(name="gath", bufs=2) as gathp,
            tc.tile_pool(name="const", bufs=1) as constp,
            tc.tile_pool(name="idxraw", bufs=2) as idxrawp,
            tc.tile_pool(name="dense", bufs=4) as densep,
            tc.tile_pool(name="tcol", bufs=2) as tcolp,
            tc.tile_pool(name="work", bufs=1) as workp,
            tc.tile_pool(name="accp", bufs=1) as accp,
            tc.tile_pool(name="psum", bufs=1, space="PSUM") as psump,
            tc.tile_pool(name="dramded", bufs=6, space="DRAM") as dedp,
        ):
            # ---------------- constants ------------------------------------
            cst = constp.tile([128, C_END], F32)
            nc.sync.dma_start(
                out=cst[:],
                in_=feat.ap()[NS:NS + 2].rearrange("a f -> (a f)").rearrange(
                    "(p f) -> p f", f=CPP)[:, 0:C_END],
            )
            mT = cst[:, C_MT:C_MT + DP]
            shB = cst[:, C_SHB:C_SHB + DP]
            shA = cst[:, C_SHA:C_SHA + DP]
            btab = cst[:, C_BT:C_BT + 30]
            atab = cst[:, C_AT:C_AT + 26]
            traw = cst[:, C_TR:C_TR + 50]
            multf = cst[:, C_MU:C_MU + 25]
            opt3 = cst[:, C_OP:C_OP + 3]
            blk = cst[:, C_BLK:C_BLK + 8]
            tv25 = cst[:, C_TV:C_TV + 25]

            fwork = constp.tile([128, 136], F32)
            cb = fwork[:, 0:4]
            nc.vector.memset(cb[:, 0:1], EPS)
            nc.vector.memset(cb[:, 1:2], PI / 2.0)
            nc.vector.memset(cb[:, 2:3], 1e-30)
            nc.vector.memset(cb[:, 3:4], -1.0)
            b_eps = cb[:, 0:1]
            b_pi2 = cb[:, 1:2]
            b_tiny = cb[:, 2:3]
            s_neg1 = cb[:, 3:4]

            # torsion derived table (k, cos d, sin d, n) x 25, on all parts
            ttab = fwork[:, 4:104]
            t4v = ttab.rearrange("p (n d) -> p n d", d=4)
            trv = traw.rearrange("p (n d) -> p n d", d=2)
            nc.vector.tensor_copy(out=t4v[:, :, 0], in_=trv[:, :, 0])          # k
            # cos d = sin(pi/2 - d); d in [0, 3.15) keeps the arg in [-pi, pi]
            carg = fwork[:, 110:135]
            nc.vector.tensor_scalar(out=carg, in0=trv[:, :, 1], scalar1=-1.0,
                                    scalar2=PI / 2.0, op0=Op.mult, op1=Op.add)
            nc.scalar.activation(t4v[:, :, 1], carg, AF.Sin)
            nc.scalar.activation(t4v[:, :, 2], trv[:, :, 1], AF.Sin)           # sin d
            nc.vector.tensor_copy(out=t4v[:, :, 3], in_=multf)                 # n

            opt6 = fwork[0:8, 104:110]
            nc.vector.tensor_copy(out=opt6[:, 0:3], in_=opt3[0:8, :])
            nc.vector.tensor_copy(out=opt6[:, 3:6], in_=opt3[0:8, :])

            accb = accp.tile([128, DP + 6 + 8], F32)
            scr = accb[:, 0:DP]            # TTR mandatory elementwise out
            acc6 = accb[:, DP:DP + 6]
            otmp = accb[0:8, DP + 6:DP + 12]
            rtmp = accb[:, DP + 12:DP + 13]

            btv = btab.rearrange("p (n d) -> p n d", d=2)
            atv = atab.rearrange("p (n d) -> p n d", d=2)
            ttab4 = ttab.rearrange("p (n d) -> p n d", d=4)

            # ========== PREP PHASE: both waves' staging + index prep =====
            # (emitted before any energy math so wave-1 gather prerequisites
            # are not queued behind wave-0 energy on the DVE)
            wave_state = []
            for w in range(2):
                iraw = idxrawp.tile([128, 3072], F32, tag="iraw")
                braw = iraw[:, 0:768]
                araw = iraw[:, 768:1792]
                trawi = iraw[:, 1792:3072]
                rows = feat.ap()[8 * w:8 * w + 8]
                nc.sync.dma_start(
                    out=braw,
                    in_=rows[:, OFF6:OFF6 + 12288].rearrange(
                        "a (j f) -> a j f", j=16),
                )
                nc.sync.dma_start(
                    out=araw,
                    in_=rows[:, OFF7:OFF7 + 16384].rearrange(
                        "a (j f) -> a j f", j=16),
                )
                nc.sync.dma_start(
                    out=trawi,
                    in_=rows[:, OFF8:OFF8 + 20480].rearrange(
                        "a (j f) -> a j f", j=16),
                )

                idxt = idx16p.tile([128, 10 * 256], I16, tag="idxt")
                bv = braw.rearrange("p (e k) -> p e k", k=3)
                av = araw.rearrange("p (e k) -> p e k", k=4)
                tv = trawi.rearrange("p (e k) -> p e k", k=5)
                for k in range(2):
                    nc.vector.tensor_copy(out=idxt[:, 256 * k:256 * (k + 1)],
                                          in_=bv[:, :, k])
                for k in range(3):
                    nc.vector.tensor_copy(out=idxt[:, 256 * (2 + k):256 * (3 + k)],
                                          in_=av[:, :, k])
                for k in range(5):
                    nc.vector.tensor_copy(out=idxt[:, 256 * (5 + k):256 * (6 + k)],
                                          in_=tv[:, :, k])
                # lists: 0,1 = bond i,j; 2..4 = angle i,j,k; 5..9 = tor i,j,k,l,t

                tcols = tcolp.tile([128, 2 * DP], F32, tag="tcols")
                tB = tcols[:, 0:DP]
                tA = tcols[:, DP:2 * DP]
                nc.vector.tensor_tensor(out=tB, in0=bv[:, :, 2], in1=shB, op=Op.add)
                nc.vector.tensor_tensor(out=tA, in0=av[:, :, 3], in1=shA, op=Op.add)
                wave_state.append((idxt, tB, tA))

            # ========== COMPUTE PHASE: per-wave gathers + energy ==========
            for w in range(2):
                idxt, tB, tA = wave_state[w]

                def idx_list(n):
                    return idxt[:, 256 * n:256 * (n + 1)]

                # gather table (replicated coords): partition p holds sample
                # (p//16)'s coords; step-0 src AP replicates each sample's
                # row across its 16 partitions
                table = tablep.tile([128, 3 * NATOMS], F32, tag="table")
                nc.sync.dma_start(
                    out=table[:],
                    in_=feat.ap()[8 * w:8 * w + 8][:, None, OFF5:OFF5 + 3 * NATOMS]
                        .to_broadcast([8, 16, 3 * NATOMS]),
                )

                def gather_dedup(idx_n, tab_ap, n_elems, d):
                    """ap_gather LIST indices; dedup 16x replicas via DRAM.
                    Returns dense [128, DP*d] tile (edge position 256j+u on
                    partition 16c+j at cols u*d..). Chunk size keeps every
                    call inside the same 24KB/partition pool slot."""
                    gch = 2048 if d == 3 else 1024
                    ded = dedp.tile([8, LIST * d], F32, tag="ded")
                    for ch in range(LIST // gch):
                        g = gathp.tile([128, gch * d], F32, tag="g", name="g")
                        if ABL_NO_GATHER:
                            nc.vector.memset(g[0:8, 0:1], 0.0)
                        else:
                            nc.gpsimd.ap_gather(
                                out_ap=g[:].rearrange("p (n d) -> p n d", d=d),
                                in_ap=tab_ap,
                                idxs_ap=idx_list(idx_n)[:, (gch // 16) * ch:(gch // 16) * (ch + 1)],
                                channels=128,
                                num_elems=n_elems,
                                d=d,
                                num_idxs=gch,
                            )
                        nc.sync.dma_start(
                            out=ded[:][:, gch * d * ch:gch * d * (ch + 1)],
                            in_=g[:].rearrange("(a b) f -> a b f", b=16)[:, 0, :],
                        )
                    dn = densep.tile([128, DP * d], F32, tag="dense")
                    ded_flat = ded[:].rearrange("s f -> (s f)")
                    nc.sync.dma_start(
                        out=dn[:],
                        in_=ded_flat.rearrange("(p f) -> p f", f=DP * d),
                    )
                    return dn

                tab3 = table[:].rearrange("p (n d) -> p n d", d=3)

                acc = acc6[:, 3 * w:3 * w + 3]
                nc.vector.memset(acc, 0.0)

                def type_masked_energy(xcol, tcol, ptab, n_types, acc_col):
                    """acc_col += sum_t k_t sum_{e: t(e)=t} (x_e - x0_t)^2,
                    processed 4 types per batch with broadcast APs."""
                    if ABL_NO_TYPES:
                        nc.vector.tensor_reduce(out=rtmp, in_=xcol, axis=AX.X, op=Op.add)
                        nc.vector.tensor_add(out=acc_col, in0=acc_col, in1=rtmp)
                        return
                    st = workp.tile([128, 32], F32, tag="wst")
                    S = st[:, 0:n_types]
                    wide = workp.tile([128, 9 * DP], F32, tag="w9")
                    for t0 in range(0, n_types, 4):
                        nb = min(4, n_types - t0)
                        msk = wide[:, 0:nb * DP].rearrange(
                            "p (t e) -> p t e", e=DP)
                        dif = wide[:, 4 * DP:(4 + nb) * DP].rearrange(
                            "p (t e) -> p t e", e=DP)
                        nc.vector.tensor_tensor(
                            out=msk,
                            in0=tcol[:, None, :].to_broadcast([128, nb, DP]),
                            in1=tv25[:, t0:t0 + nb][:, :, None].to_broadcast(
                                [128, nb, DP]),
                            op=Op.is_equal)
                        nc.vector.tensor_tensor(
                            out=dif,
                            in0=xcol[:, None, :].to_broadcast([128, nb, DP]),
                            in1=ptab[:, t0:t0 + nb, 1][:, :, None].to_broadcast(
                                [128, nb, DP]),
                            op=Op.subtract)
                        nc.scalar.activation(dif, dif, AF.Square)
                        nc.vector.tensor_tensor(out=msk, in0=msk, in1=dif,
                                                op=Op.mult)
                        nc.vector.tensor_reduce(
                            out=S[:, t0:t0 + nb], in_=msk, axis=AX.X, op=Op.add)
                    cmb = workp.tile([128, 32], F32, tag="wcmb")
                    c1 = cmb[:, 0:n_types]
                    nc.vector.tensor_tensor(out=c1, in0=S, in1=ptab[:, 0:n_types, 0],
                                            op=Op.mult)
                    nc.vector.tensor_reduce(out=rtmp, in_=c1, axis=AX.X, op=Op.add)
                    nc.vector.tensor_add(out=acc_col, in0=acc_col, in1=rtmp)

                # ==================== BONDS ====================
                ci = gather_dedup(0, tab3, NATOMS, 3)
                cj = gather_dedup(1, tab3, NATOMS, 3)
                d3 = workp.tile([128, 3 * DP], F32, tag="w3a")
                nc.vector.tensor_sub(out=d3[:], in0=ci[:], in1=cj[:])
                d3s = workp.tile([128, 3 * DP], F32, tag="w3b")
                nc.vector.tensor_mul(out=d3s[:], in0=d3[:], in1=d3[:])
                wb = workp.tile([128, 8 * DP], F32, tag="w8")
                r2 = wb[:, 0:DP]
                nc.vector.tensor_reduce(
                    out=r2, in_=d3s[:].rearrange("p (n d) -> p n d", d=3),
                    axis=AX.X, op=Op.add,
                )
                r = wb[:, DP:2 * DP]
                nc.scalar.activation(r, r2, AF.Sqrt, bias=b_eps)
                type_masked_energy(r, tB, btv, 15, acc[:, 0:1])

                # ==================== ANGLES ====================
                gi = gather_dedup(2, tab3, NATOMS, 3)
                gj = gather_dedup(3, tab3, NATOMS, 3)
                gk = gather_dedup(4, tab3, NATOMS, 3)
                v1 = workp.tile([128, 3 * DP], F32, tag="w3a")
                v2 = workp.tile([128, 3 * DP], F32, tag="w3b")
                nc.vector.tensor_sub(out=v1[:], in0=gi[:], in1=gj[:])
                nc.vector.tensor_sub(out=v2[:], in0=gk[:], in1=gj[:])
                prod = workp.tile([128, 3 * DP], F32, tag="w3c")
                wa = workp.tile([128, 8 * DP], F32, tag="w8")
                d11 = wa[:, 0:DP]
                d22 = wa[:, 1 * DP:2 * DP]
                d12 = wa[:, 2 * DP:3 * DP]

                def dot3(dst, a, b):
                    nc.vector.tensor_mul(out=prod[:], in0=a[:], in1=b[:])
                    nc.vector.tensor_reduce(
                        out=dst, in_=prod[:].rearrange("p (n d) -> p n d", d=3),
                        axis=AX.X, op=Op.add,
                    )

                dot3(d11, v1, v1)
                dot3(d22, v2, v2)
                dot3(d12, v1, v2)
                s1 = wa[:, 3 * DP:4 * DP]
                s2a = wa[:, 4 * DP:5 * DP]
                nc.scalar.activation(s1, d11, AF.Sqrt, bias=b_eps)
                nc.scalar.activation(s2a, d22, AF.Sqrt, bias=b_eps)
                den = wa[:, 5 * DP:6 * DP]
                nc.vector.tensor_mul(out=den, in0=s1, in1=s2a)
                cosv = wa[:, 6 * DP:7 * DP]
                nc.vector.reciprocal(out=den, in_=den)
                nc.vector.tensor_mul(out=cosv, in0=d12, in1=den)
                cosc = wa[:, 7 * DP:8 * DP]
                nc.vector.tensor_scalar(
                    out=cosc, in0=cosv, scalar1=-1.0 + 1e-6, scalar2=1.0 - 1e-6,
                    op0=Op.max, op1=Op.min,
                )
                # theta = arccos(cosc) via two bounded-arg arctan branches
                # (ACT Arctan domain is [-pi/2, pi/2] so |arg| <= 1 required):
                #  |c| >  s: theta = arctan(s/c) + pi*(c<0)
                #  |c| <= s: theta = pi/2 - arctan(c/s), s = sqrt(1-c^2)
                cc = wa[:, 0:DP]                       # d11 dead
                nc.scalar.activation(cc, cosc, AF.Square)
                om = wa[:, 1 * DP:2 * DP]              # d22 dead
                nc.vector.tensor_scalar(
                    out=om, in0=cc, scalar1=-1.0, scalar2=1.0, op0=Op.mult, op1=Op.add
                )
                sn = wa[:, 2 * DP:3 * DP]              # d12 dead
                nc.scalar.activation(sn, om, AF.Sqrt)
                sgn = wa[:, 3 * DP:4 * DP]             # s1 dead
                nc.vector.tensor_scalar(
                    out=sgn, in0=cosc, scalar1=0.0, scalar2=None, op0=Op.is_ge)
                nc.vector.tensor_scalar(
                    out=sgn, in0=sgn, scalar1=2e-18, scalar2=-1e-18,
                    op0=Op.mult, op1=Op.add)
                csafe = wa[:, 4 * DP:5 * DP]           # s2a dead
                nc.vector.tensor_add(out=csafe, in0=cosc, in1=sgn)
                ra = wa[:, 3 * DP:4 * DP]              # sgn dead
                nc.vector.reciprocal(out=csafe, in_=csafe)
                nc.vector.tensor_mul(out=ra, in0=sn, in1=csafe)
                nc.vector.tensor_scalar(
                    out=ra, in0=ra, scalar1=-1.0, scalar2=1.0, op0=Op.max, op1=Op.min)
                ata = wa[:, 4 * DP:5 * DP]             # csafe dead
                nc.scalar.activation(ata, ra, AF.Arctan)
                corr = wa[:, 5 * DP:6 * DP]            # den dead
                nc.vector.tensor_scalar(
                    out=corr, in0=cosc, scalar1=0.0, scalar2=PI, op0=Op.is_lt, op1=Op.mult
                )
                tha = wa[:, 3 * DP:4 * DP]             # ra dead
                nc.vector.tensor_add(out=tha, in0=ata, in1=corr)
                rb = wa[:, 4 * DP:5 * DP]              # ata dead
                nc.vector.reciprocal(out=sn, in_=sn)
                nc.vector.tensor_mul(out=rb, in0=cosc, in1=sn)
                nc.vector.tensor_scalar(
                    out=rb, in0=rb, scalar1=-1.0, scalar2=1.0, op0=Op.max, op1=Op.min)
                thb = wa[:, 5 * DP:6 * DP]             # corr dead
                nc.scalar.activation(thb, rb, AF.Arctan)
                nc.vector.tensor_scalar(
                    out=thb, in0=thb, scalar1=-1.0, scalar2=PI / 2.0,
                    op0=Op.mult, op1=Op.add)
                wi2 = workp.tile([128, DP], I32, tag="wi")
                mbr = wi2[:, 0:DP]
                nc.vector.tensor_scalar(
                    out=mbr, in0=cc, scalar1=0.5, scalar2=None, op0=Op.is_gt)
                th = wa[:, 6 * DP:7 * DP]              # cosv dead
                nc.vector.select(out=th, mask=mbr, on_true=tha, on_false=thb)
                type_masked_energy(th, tA, atv, 13, acc[:, 1:2])

                # ==================== TORSIONS ====================
                ti = gather_dedup(5, tab3, NATOMS, 3)
                tj = gather_dedup(6, tab3, NATOMS, 3)
                tk_ = gather_dedup(7, tab3, NATOMS, 3)
                tl = gather_dedup(8, tab3, NATOMS, 3)
                b1 = workp.tile([128, 3 * DP], F32, tag="w3a")
                b2 = workp.tile([128, 3 * DP], F32, tag="w3b")
                b3 = workp.tile([128, 3 * DP], F32, tag="w3c")
                nc.vector.tensor_sub(out=b1[:], in0=tj[:], in1=ti[:])
                nc.vector.tensor_sub(out=b2[:], in0=tk_[:], in1=tj[:])
                nc.vector.tensor_sub(out=b3[:], in0=tl[:], in1=tk_[:])
                pt = gather_dedup(9, ttab4, 25, 4)
                pl = workp.tile([128, 9 * DP], F32, tag="w9")

                def plv(n):
                    return pl[:, DP * n:DP * (n + 1)]

                for m in range(3):
                    nc.vector.tensor_copy(
                        out=plv(0 + m),
                        in_=b1[:].rearrange("p (n d) -> p n d", d=3)[:, :, m])
                    nc.vector.tensor_copy(
                        out=plv(3 + m),
                        in_=b2[:].rearrange("p (n d) -> p n d", d=3)[:, :, m])
                    nc.vector.tensor_copy(
                        out=plv(6 + m),
                        in_=b3[:].rearrange("p (n d) -> p n d", d=3)[:, :, m])
                # n1 = b1 x b2 -> cr 0..2 ; n2 = b2 x b3 -> cr 3..5
                cr_ = workp.tile([128, 6 * DP], F32, tag="w6")

                def crv(n):
                    return cr_[:, DP * n:DP * (n + 1)]

                tmp = workp.tile([128, 2 * DP], F32, tag="w2")
                t0 = tmp[:, 0:DP]
                t1_ = tmp[:, DP:2 * DP]
                for m in range(3):
                    mp1, mp2 = (m + 1) % 3, (m + 2) % 3
                    nc.vector.tensor_mul(out=t0, in0=plv(0 + mp1), in1=plv(3 + mp2))
                    nc.vector.tensor_mul(out=t1_, in0=plv(0 + mp2), in1=plv(3 + mp1))
                    nc.vector.tensor_sub(out=crv(m), in0=t0, in1=t1_)
                    nc.vector.tensor_mul(out=t0, in0=plv(3 + mp1), in1=plv(6 + mp2))
                    nc.vector.tensor_mul(out=t1_, in0=plv(3 + mp2), in1=plv(6 + mp1))
                    nc.vector.tensor_sub(out=crv(3 + m), in0=t0, in1=t1_)
                wt = workp.tile([128, 8 * DP], F32, tag="w8")
                q2 = wt[:, 0:DP]
                nc.vector.tensor_mul(out=b1[:], in0=b2[:], in1=b2[:])  # b1 = scratch
                nc.vector.tensor_reduce(
                    out=q2, in_=b1[:].rearrange("p (n d) -> p n d", d=3),
                    axis=AX.X, op=Op.add,
                )
                # m1' = n1 x b2 (normalization folded into rn)
                mp = workp.tile([128, 3 * DP], F32, tag="w3a")

                def mpv(n):
                    return mp[:, DP * n:DP * (n + 1)]

                for m in range(3):
                    mp1, mp2 = (m + 1) % 3, (m + 2) % 3
                    nc.vector.tensor_mul(out=t0, in0=crv(mp1), in1=plv(3 + mp2))
                    nc.vector.tensor_mul(out=t1_, in0=crv(mp2), in1=plv(3 + mp1))
                    nc.vector.tensor_sub(out=mpv(m), in0=t0, in1=t1_)
                X = wt[:, 1 * DP:2 * DP]
                Y = wt[:, 2 * DP:3 * DP]
                nc.vector.tensor_mul(out=t0, in0=crv(0), in1=crv(3))
                nc.vector.tensor_mul(out=t1_, in0=crv(1), in1=crv(4))
                nc.vector.tensor_add(out=X, in0=t0, in1=t1_)
                nc.vector.tensor_mul(out=t0, in0=crv(2), in1=crv(5))
                nc.vector.tensor_add(out=X, in0=X, in1=t0)
                nc.vector.tensor_mul(out=t0, in0=mpv(0), in1=crv(3))
                nc.vector.tensor_mul(out=t1_, in0=mpv(1), in1=crv(4))
                nc.vector.tensor_add(out=Y, in0=t0, in1=t1_)
                nc.vector.tensor_mul(out=t0, in0=mpv(2), in1=crv(5))
                nc.vector.tensor_add(out=Y, in0=Y, in1=t0)
                rn = wt[:, 3 * DP:4 * DP]
                nc.scalar.activation(rn, q2, AF.Sqrt, bias=b_eps)
                y = wt[:, 4 * DP:5 * DP]
                nc.vector.reciprocal(out=rn, in_=rn)
                nc.vector.tensor_mul(out=y, in0=Y, in1=rn)
                hx = wt[:, 5 * DP:6 * DP]
                hy = wt[:, 6 * DP:7 * DP]
                nc.scalar.activation(hx, X, AF.Square)
                nc.scalar.activation(hy, y, AF.Square)
                h = wt[:, 7 * DP:8 * DP]
                nc.vector.tensor_add(out=h, in0=hx, in1=hy)
                rh = wt[:, 5 * DP:6 * DP]              # hx dead
                nc.scalar.activation(rh, h, AF.Sqrt, bias=b_tiny)
                c = wt[:, 0:DP]                        # q2 dead
                s = wt[:, 6 * DP:7 * DP]               # hy dead
                nc.vector.reciprocal(out=rh, in_=rh)
                nc.vector.tensor_mul(out=c, in0=X, in1=rh)
                nc.vector.tensor_mul(out=s, in0=y, in1=rh)
                # Chebyshev: cos/sin of 2phi and 3phi (reuse pl slices: b1/b3
                # component planes are dead after the cross products)
                cc_ = plv(0)
                c2 = plv(1)
                s2 = plv(2)
                c3 = plv(6)
                s3 = plv(7)
                sc = plv(8)
                nc.scalar.activation(cc_, c, AF.Square)
                nc.vector.tensor_scalar(
                    out=c2, in0=cc_, scalar1=2.0, scalar2=-1.0, op0=Op.mult, op1=Op.add)
                nc.vector.tensor_mul(out=sc, in0=s, in1=c)
                nc.vector.tensor_scalar(
                    out=s2, in0=sc, scalar1=2.0, scalar2=None, op0=Op.mult)
                nc.vector.tensor_scalar(
                    out=t0, in0=cc_, scalar1=4.0, scalar2=-3.0, op0=Op.mult, op1=Op.add)
                nc.vector.tensor_mul(out=c3, in0=t0, in1=c)
                nc.vector.tensor_scalar(
                    out=t0, in0=cc_, scalar1=4.0, scalar2=-1.0, op0=Op.mult, op1=Op.add)
                nc.vector.tensor_mul(out=s3, in0=t0, in1=s)
                ptv = pt[:].rearrange("p (n d) -> p n d", d=4)
                wi3 = workp.tile([128, 2 * DP], I32, tag="wi2")
                m2m = wi3[:, 0:DP]
                m3m = wi3[:, DP:2 * DP]
                nc.vector.tensor_scalar(
                    out=m2m, in0=ptv[:, :, 3], scalar1=2.0, scalar2=None, op0=Op.is_equal)
                nc.vector.tensor_scalar(
                    out=m3m, in0=ptv[:, :, 3], scalar1=3.0, scalar2=None, op0=Op.is_equal)
                cn = wt[:, 3 * DP:4 * DP]              # rn dead
                sn2 = wt[:, 4 * DP:5 * DP]             # y dead
                nc.vector.select(out=cn, mask=m2m, on_true=c2, on_false=c)
                nc.vector.select(out=cn, mask=m3m, on_true=c3, on_false=cn)
                nc.vector.select(out=sn2, mask=m2m, on_true=s2, on_false=s)
                nc.vector.select(out=sn2, mask=m3m, on_true=s3, on_false=sn2)
                tt1 = wt[:, 5 * DP:6 * DP]             # rh dead
                tt2 = wt[:, 6 * DP:7 * DP]             # s dead (selects done)
                nc.vector.tensor_mul(out=tt1, in0=cn, in1=ptv[:, :, 1])
                nc.vector.tensor_mul(out=tt2, in0=sn2, in1=ptv[:, :, 2])
                esum = wt[:, 7 * DP:8 * DP]            # h dead
                nc.vector.tensor_add(out=esum, in0=tt1, in1=tt2)
                nc.vector.tensor_scalar(
                    out=esum, in0=esum, scalar1=1.0, scalar2=None, op0=Op.add)
                kmt = wt[:, 0:DP]                      # c dead
                nc.vector.tensor_tensor(out=kmt, in0=ptv[:, :, 0], in1=mT, op=Op.mult)
                nc.vector.tensor_mul(out=scr, in0=esum, in1=kmt)
                nc.vector.tensor_reduce(out=rtmp, in_=scr, axis=AX.X, op=Op.add)
                nc.vector.tensor_add(out=acc[:, 2:3], in0=acc[:, 2:3], in1=rtmp)

            # ------------- final reduction: [128, 6] -> [8, 6] -> out ------
            pacc = psump.tile([8, 6], F32, tag="pacc")
            nc.tensor.matmul(out=pacc[:], lhsT=blk, rhs=acc6, start=True, stop=True)
            nc.vector.tensor_copy(out=otmp, in_=pacc[:])
            nc.vector.tensor_mul(out=otmp, in0=otmp, in1=opt6)
            nc.sync.dma_start(out=out_d.ap()[0:8, :], in_=otmp[:, 0:3])
            nc.sync.dma_start(out=out_d.ap()[8:16, :], in_=otmp[:, 3:6])

    nc.compile()
    return nc


@functools.lru_cache(maxsize=1)
def _get_nc():
    return build_nc()


def _const_row():
    """[128, CPP] f32: per-partition constant block, see C_* layout."""
    row = np.zeros((128, CPP), np.float32)
    u = np.arange(DP)
    tail16 = np.zeros((128, 1), np.float32)
    tail16[15::16] = 1.0
    row[:, C_MT:C_MT + DP] = 1.0 - tail16 * (u >= DP - 3)[None, :]
    row[:, C_SHB:C_SHB + DP] = 999.0 * tail16 * (u >= DP - 1)[None, :]
    row[:, C_SHA:C_SHA + DP] = 999.0 * tail16 * (u >= DP - 2)[None, :]
    p = np.arange(128)
    row[:, C_BLK:C_BLK + 8] = (p[:, None] // 16 == np.arange(8)[None, :])
    row[:, C_TV:C_TV + 25] = np.arange(25)[None, :]
    return row


def make_in_maps(inputs):
    """Shard full inputs into 8 per-core single-tensor input maps.

    Pure layout: slice the 4 used feature columns (coords/bonds/angles/
    torsions) into per-sample packed rows; append 2 rows of per-partition
    constants (param tables, tail masks, selector)."""
    feats = np.asarray(inputs["features"], dtype=np.float32)
    Bf = feats.shape[0]
    rows = np.concatenate([
        feats[:, 0:12288, 5],
        feats[:, 0:12288, 6],
        feats[:, 0:16384, 7],
        feats[:, 0:20465, 8],
        np.zeros((Bf, 15), np.float32),          # pad torsion col to 20480
    ], axis=1)                                   # [B, ROWLEN]
    crow = _const_row()
    crow[:, C_BT:C_BT + 30] = np.asarray(inputs["bond_type"], np.float32).reshape(-1)
    crow[:, C_AT:C_AT + 26] = np.asarray(inputs["angle_type"], np.float32).reshape(-1)
    crow[:, C_TR:C_TR + 50] = np.asarray(inputs["tor_type"], np.float32).reshape(-1)
    crow[:, C_MU:C_MU + 25] = np.asarray(inputs["multiplicity"], np.float32)
    crow[:, C_OP:C_OP + 3] = np.asarray(inputs["opt_pars"], np.float32)[0:3]
    crow_flat = crow.reshape(2, ROWLEN)
    n_nc = Bf // NS
    in_maps = []
    for k in range(n_nc):
        blob = np.concatenate([rows[NS * k:NS * (k + 1)], crow_flat], axis=0)
        in_maps.append({"features": np.ascontiguousarray(blob)})
    return in_maps


def kernel(**inputs) -> np.ndarray:
    from concourse.bass_utils import run_bass_kernel_spmd

    nc = _get_nc()
    in_maps = make_in_maps(inputs)
    res = run_bass_kernel_spmd(nc, in_maps, core_ids=list(range(len(in_maps))))
    outs = [res.results[k]["out"] for k in range(len(in_maps))]
    return np.concatenate(outs, axis=0).astype(np.float32)


def simulate_one_core(inputs, nc=None):
    """CoreSim a single NC on the first 16 samples (for correctness dev)."""
    import concourse.bass_interp as bass_interp

    if nc is None:
        nc = _get_nc()
    in_map = make_in_maps(inputs)[0]
    sim = bass_interp.MultiCoreSim(nc, 1)
    for name, val in in_map.items():
        sim.cores[0].tensor(name)[:] = val
    sim.simulate(check_with_hw=False)
    return np.array(sim.cores[0].mem_tensor("out"))


if __name__ == "__main__":
    nc = build_nc()
    print("build ok")
